# revision 8
# baseline (speedup 1.0000x reference)
"""Chamfer distance + F1 kernel for Trainium2 (8 NeuronCores).

Banded-KNN strategy (B=4 batches, N=M=8192 points, 3D):
  - Host sorts each batch's clouds by radius r=||p||.  Since radius is
    1-Lipschitz (|r_x - r_y| <= ||x-y||), a point's nearest neighbour is
    radially close whenever it is close in space, so a narrow band around
    the radius-sorted diagonal contains the NN for all but a handful of
    isolated points.
  - core c handles batch b = c//2, sorted-row-half h = c%2 (4096 rows).
    Each 128-row tile t computes scaled squared distances only against a
    static W-wide window of radius-sorted xyz2 (window slides 128 cols per
    tile).  The host hands each core a pre-shifted, padded slice of sorted
    xyz2 so the compiled program is identical across cores (SPMD).
  - PE computes the W-wide block in fp16 via the augmented K=13 contraction
    (fp32 operands hi/lo-split into fp16), ACT converts PSUM fp32 -> SBUF
    fp16 with Relu, DVE does the banded row-min (fused custom MINMIN op at
    2x) and a sliding col-min accumulator M (fp16 tensor_tensor at 2x).
    PE transposes finished M blocks mid-loop, DVE reduces -> col mins.
  - Host patch: points whose banded min exceeds the window's radial
    coverage (provable criterion) get exact numpy rows (~35/batch-side),
    making the result exact up to fp16 rounding.  cd/f1 host finalize.
"""

import sys

if "/opt/trn_rl_repo" not in sys.path:
    sys.path.insert(0, "/opt/trn_rl_repo")

from contextlib import ExitStack

import numpy as np

import concourse.tile as tile
import concourse.dve_ops as dve_ops
from concourse import bacc, bass_isa, mybir
from concourse.bass_utils import run_bass_kernel_spmd
from concourse.dve_spec import C0, AluOp, Spec, Src0, Src1, lower, minn
from concourse.dve_uop import DveOpSpec

F16 = mybir.dt.float16
F32 = mybir.dt.float32
MIN = mybir.AluOpType.min
AXX = mybir.AxisListType.X

SCALE = 24.0  # coordinate prescale; distances come out scaled by SCALE**2
DSCALE = SCALE * SCALE
F1_THRESHOLD = 1e-4
PAD_DIST = 60000.0  # scaled distance of padding columns (> max real ~53k)

N_CORES = 8
K_AUG = 13  # 9 coord-product rows + 2 sq1 rows + 2 sq2 rows
BAND_W = 768  # band width (columns per row-tile window)
USE_MINMIN_2X = False

_RealCDA = bass_isa.InstCustomDveAnt


def _cda_perf(*a, **kw):
    """bass.py constructs InstCustomDveAnt via module attr; force perf_max=1
    (2x_1PORT opt-in, instruction byte 36[7:6]) for the MINMIN op."""
    inst = _RealCDA(*a, **kw)
    if USE_MINMIN_2X and kw.get("op_name") == "MINMIN_REDUCE_ANT":
        inst.perf_max = 1
    return inst


bass_isa.InstCustomDveAnt = _cda_perf


def _split16(v):
    """Split fp32 array into fp16 hi + lo so hi+lo ~= v to ~2^-22 rel."""
    hi = v.astype(np.float16)
    lo = (v - hi.astype(np.float32)).astype(np.float16)
    return hi, lo


def _aug_rows(xyz, side):
    """Augmented fp16 operand rows [K_AUG, n] for one cloud.

    side='lhs': rows for xyz1 (stationary), side='rhs': rows for xyz2
    (moving), such that sum_k lhsT[k,p]*rhs[k,f] ~= DSCALE*||x1_p - x2_f||^2.
    """
    n = xyz.shape[0]
    out = np.empty((K_AUG, n), np.float16)
    if side == "lhs":
        v = (-2.0 * SCALE) * xyz.astype(np.float32)
        h, l = _split16(v)
        sq = ((SCALE * xyz.astype(np.float32)) ** 2).sum(axis=1) * 0.25
        sh, sl = _split16(sq)
        for c in range(3):
            out[3 * c + 0] = h[:, c]
            out[3 * c + 1] = h[:, c]
            out[3 * c + 2] = l[:, c]
        out[9] = sh
        out[10] = sl
        out[11] = np.float16(4.0)
        out[12] = np.float16(4.0)
    else:
        w = SCALE * xyz.astype(np.float32)
        h, l = _split16(w)
        sq = ((w.astype(np.float64) ** 2).sum(axis=1) * 0.25).astype(np.float32)
        sh, sl = _split16(sq)
        for c in range(3):
            out[3 * c + 0] = h[:, c]
            out[3 * c + 1] = l[:, c]
            out[3 * c + 2] = h[:, c]
        out[9] = np.float16(4.0)
        out[10] = np.float16(4.0)
        out[11] = sh
        out[12] = sl
    return out


def _build_minmin_2x(uops_1x):
    """Hand-authored 2x_1PORT uop program for MINMIN (no stock accum op runs
    above 1x; this processes 2 packed fp16 pairs per port per cycle).

    steady-state, per cycle:
      lanes: 1=SRC_0 2=SRC_1 3=SRC_0_HI 4=SRC_1_HI
      s0: lo = MIN(SRC_0, SRC_1); delay lanes carry the HI pair forward
      s1: hi = MIN(SRC_0_HI, SRC_1_HI); delay0 captures s0's lo (realign)
      s2: merged = MIN(hi[curr], lo[delay0])
      s3: acc = MIN(merged[curr], acc[self])  (alu_out_a = accumulator)
      s4-7: hold.  out tensor gets scratch values; only accum_out is used.
    """
    from copy import deepcopy

    from concourse.dve_uop import (
        AluInp, DelayInp, InpSel, OutPath, OutSel, Trigger,
        UopConfig, UopDpConfig,
    )
    from concourse.dve_uop import AluOp as UAluOp

    PD = DelayInp.PREV_DELAY
    PAO = DelayInp.PREV_ALU_OUT

    def dp(op, s0, s1, a=0):
        return UopDpConfig(
            op=op, alu_src0=s0, alu_src1=s1,
            delay=[PD, PD, PD, PD, PAO, PAO, PAO],
            alu_out_enable=1, swap_enable=0,
            alu_out_a_enable=a, alu_out_b_enable=0,
            delay_enable=[1, 0, 0, 0, 0, 0, 0],
            idx0_sel=0, idx1_sel=0,
        )

    s0 = dp(UAluOp.MIN, AluInp.PREV_DELAY_0, AluInp.PREV_DELAY_1)
    s0.delay_enable = [1, 1, 1, 1, 0, 0, 0]
    s1 = dp(UAluOp.MIN, AluInp.PREV_DELAY_2, AluInp.PREV_DELAY_3)
    s1.delay = [PAO, PD, PD, PD, PAO, PAO, PAO]
    stages = [
        s0,
        s1,
        # s2: PREV_ALU_OUT = s1's hi (prev block), PREV_DELAY_0 = lo
        dp(UAluOp.MIN, AluInp.PREV_ALU_OUT, AluInp.PREV_DELAY_0),
        # s3: CURR_ALU_OUT = own flop = the accumulator (seeded with C0)
        dp(UAluOp.MIN, AluInp.CURR_ALU_OUT, AluInp.PREV_ALU_OUT, a=1),
        dp(UAluOp.BYPASS, AluInp.PREV_ALU_OUT, AluInp.PREV_ALU_OUT, a=1),
        dp(UAluOp.BYPASS, AluInp.PREV_ALU_OUT, AluInp.PREV_ALU_OUT, a=1),
        dp(UAluOp.BYPASS, AluInp.PREV_ALU_OUT, AluInp.PREV_ALU_OUT, a=1),
        dp(UAluOp.BYPASS, AluInp.PREV_ALU_OUT, AluInp.PREV_ALU_OUT, a=1),
    ]

    def seed_dp():
        # seed token (1 cycle): C0 enters on input lane 3, rides delay
        # lane 2 to stage 3 which loads the accumulator flop; stage 0
        # bypasses C0 so stage 1's delay lane 0 (the merge operand in the
        # steady state) also starts at C0 instead of stale garbage.
        st0 = dp(UAluOp.BYPASS, AluInp.PREV_DELAY_2, AluInp.PREV_DELAY_2)
        st0.delay_enable = [1, 1, 1, 0, 0, 0, 0]
        st1 = dp(UAluOp.BYPASS, AluInp.PREV_ALU_OUT, AluInp.PREV_ALU_OUT)
        st1.delay = [PAO, PD, PD, PD, PAO, PAO, PAO]
        st1.delay_enable = [1, 0, 1, 0, 0, 0, 0]
        st2 = dp(UAluOp.BYPASS, AluInp.PREV_ALU_OUT, AluInp.PREV_ALU_OUT)
        st2.delay_enable = [1, 0, 1, 0, 0, 0, 0]
        st3 = dp(UAluOp.BYPASS, AluInp.PREV_DELAY_2, AluInp.PREV_DELAY_2, a=1)
        return [st0, st1, st2, st3] + [
            dp(UAluOp.BYPASS, AluInp.PREV_ALU_OUT, AluInp.PREV_ALU_OUT, a=1)
            for _ in range(4)
        ]
    steady = UopConfig(
        inp=[InpSel.ZERO, InpSel.SRC_0, InpSel.SRC_1, InpSel.SRC_0_HI,
             InpSel.SRC_1_HI, InpSel.ZERO, InpSel.ZERO, InpSel.ZERO],
        inp_enable=[0, 1, 1, 1, 1, 0, 0, 0],
        out={OutPath.WR0_LO: OutSel.ALU_OUT, OutPath.WR0_HI: OutSel.DELAY_0,
             OutPath.WR1_LO: OutSel.ALU_OUT, OutPath.WR1_HI: OutSel.ALU_OUT},
        out_enable={OutPath.WR0_LO: 1, OutPath.WR0_HI: 1,
                    OutPath.WR1_LO: 0, OutPath.WR1_HI: 0},
        out_last_subdim_enable=0,
        force_two_data_zero=0, force_two_data_one=0,
        require_inp0=1, require_inp1=1,
        repeat_count=0,
        trigger=(Trigger.SRC_TENSOR_DONE, Trigger.NONE, Trigger.NONE),
        next_uop=(0, 0, 0),
        inc_parameter_index=0, enable_rev_ops=0,
        match_mask=0, valid_match=0, replace_on_match=0, clear_match=0,
        write_predicate_select=0, write_predicate_enable=0,
        delay_shift8=0, index_increment=0, index_clear=0,
        accum_enabled=1, v4={},
        datapath_config=stages,
    )
    seed = deepcopy(uops_1x[0])
    seed.datapath_config = seed_dp()
    return [seed, steady]


def _register_minmin_reduce():
    """Custom DVE op: out = min(in0, in1); accum_out = min-fold(out)."""
    name = "MINMIN_REDUCE_ANT"
    if name in dve_ops._SUB_OPCODE_FOR_NAME:
        return next(op for op in dve_ops.OPS if op.name == name)

    def _ref(in0, in1, c0, c1, c2):
        out = np.minimum(np.asarray(in0, np.float32), np.asarray(in1, np.float32))
        acc = out.reshape(out.shape[0], -1).min(axis=-1, keepdims=True)
        acc = np.minimum(acc, c0)
        return out, acc

    spec = Spec(body=minn(Src0, Src1), accum=AluOp.MIN, accum_init=C0,
                reference=_ref)
    row = max(dve_ops._SUB_OPCODE_FOR_NAME.values()) + 1
    u1 = lower(spec, ver="v3")
    s3 = DveOpSpec(name=name, opcode=row, uops=u1,
                   uops_2x=_build_minmin_2x(u1) if USE_MINMIN_2X else None,
                   rd1_en=True, perf_max=1 if USE_MINMIN_2X else 0)
    s3.validate("v3")
    shas = {"v3": s3.sha("v3")}
    try:
        u1v4 = lower(spec, ver="v4")
        s4 = DveOpSpec(name=name, opcode=row, uops=u1v4, rd1_en=True)
        shas["v4"] = s4.sha("v4")
        dve_ops._COMPILE_CACHE[(name, "v4")] = s4
    except Exception:
        pass
    op = dve_ops.DveOp(name, spec, subdim=False, uops_sha=shas)
    dve_ops._COMPILE_CACHE[(name, "v3")] = s3
    dve_ops.OPS.append(op)
    dve_ops.CUSTOM_DVE_SPECS[name] = spec
    dve_ops._SUB_OPCODE_FOR_NAME[name] = row
    return op


def build_program(n_rows=4096, W=BAND_W):
    """Per-core banded program (SPMD-identical across cores).

    Local column space C = n_rows + W; tile t (128 rows) sees window
    [128*t, 128*t + W).  The host pre-shifts/pads each core's rhs so this
    static window pattern is centred on the tile's radius range.
    """
    ROWT = n_rows // 128
    C = n_rows + W
    COV = 128 * (ROWT - 1) + W  # local cols ever covered by a window
    NB = (COV + 127) // 128  # 128-col blocks for the transpose tail
    PER = 8  # transpose blocks per PSUM tile / reduce group
    PSW = 1024  # psum tile width (bank aligned); only first W cols used

    MINMIN = _register_minmin_reduce()
    nc = bacc.Bacc("TRN2", target_bir_lowering=False, debug=False,
                   num_devices=N_CORES)
    lhsT_d = nc.dram_tensor("lhsT", [K_AUG, n_rows], F16, kind="ExternalInput").ap()
    rhs_d = nc.dram_tensor("rhs", [K_AUG, C], F16, kind="ExternalInput").ap()
    id_d = nc.dram_tensor("ident", [128, 128], F16, kind="ExternalInput").ap()
    out1_d = nc.dram_tensor("out1", [128, ROWT], F32, kind="ExternalOutput").ap()
    out2_d = nc.dram_tensor("out2", [128, NB], F32, kind="ExternalOutput").ap()

    with tile.TileContext(nc) as tc, ExitStack() as ctx:
        const = ctx.enter_context(tc.tile_pool(name="const", bufs=1))
        d16p = ctx.enter_context(tc.tile_pool(name="d16", bufs=4))
        mp = ctx.enter_context(tc.tile_pool(name="macc", bufs=1))
        treep = ctx.enter_context(tc.tile_pool(name="tree", bufs=2))
        outp = ctx.enter_context(tc.tile_pool(name="outs", bufs=1))
        psp = ctx.enter_context(tc.tile_pool(name="ps", bufs=3, space="PSUM"))
        pstp = ctx.enter_context(tc.tile_pool(name="psT", bufs=2, space="PSUM"))

        M = mp.tile([128, C], F16)
        R = outp.tile([128, ROWT], F32)
        C2 = outp.tile([128, NB], F32)

        # init col-min accumulator while input DMAs are in flight
        nc.vector.memset(M[:, 0:C // 2], PAD_DIST)
        nc.vector.memset(M[:, C // 2:C], PAD_DIST)

        # inputs: split across the two HWDGE queues (sync + scalar), in
        # first-needed-first order so the pipeline lights up early.
        w_sb = const.tile([K_AUG, n_rows], F16)
        r_sb = const.tile([K_AUG, C], F16)
        id_sb = const.tile([128, 128], F16)
        # first two tiles' worth on both queues
        nc.sync.dma_start(r_sb[:, 0:W + 256], rhs_d[:, 0:W + 256])
        nc.scalar.dma_start(w_sb[:, 0:512], lhsT_d[:, 0:512])
        # rest of lhsT on scalar queue, rest of rhs on sync queue
        s = W + 256
        while s < C:
            w = min(1536, C - s)
            nc.sync.dma_start(r_sb[:, s:s + w], rhs_d[:, s:s + w])
            s += w
        s = 512
        while s < n_rows:
            w = min(1024, n_rows - s)
            nc.scalar.dma_start(w_sb[:, s:s + w], lhsT_d[:, s:s + w])
            s += w
        nc.scalar.dma_start(id_sb[:], id_d)

        def tail_group(q):
            nblk = min(PER, NB - q * PER)
            psT = pstp.tile([128, PER * 128], F16, tag="psT")
            for j in range(nblk):
                blk = q * PER + j
                nc.tensor.matmul(
                    psT[:, 128 * j:128 * (j + 1)],
                    M[:, 128 * blk:128 * (blk + 1)],
                    id_sb[:],
                    is_transpose=True,
                    start=(j == 0), stop=(j == nblk - 1),
                )
            nc.vector.tensor_reduce(
                C2[:, PER * q:PER * q + nblk],
                psT[:, 0:nblk * 128].rearrange("p (b c) -> p b c", c=128),
                axis=AXX, op=MIN,
            )

        for t in range(ROWT):
            lo = 128 * t
            ps = psp.tile([128, PSW], F32, tag="ps")
            for j in range((W + 511) // 512):
                wj = min(512, W - 512 * j)
                nc.tensor.matmul(
                    ps[:, 512 * j:512 * j + wj],
                    w_sb[:, 128 * t:128 * (t + 1)],
                    r_sb[:, lo + 512 * j:lo + 512 * j + wj],
                    start=True, stop=True,
                )
            d16 = d16p.tile([128, W], F16, tag="d16")
            nc.scalar.activation(
                d16[:], ps[:, 0:W], mybir.ActivationFunctionType.Relu,
            )
            # sliding col-min accumulate over this tile's window
            nc.vector.tensor_tensor(M[:, lo:lo + W], M[:, lo:lo + W],
                                    d16[:], op=MIN)
            # banded row-min via the fused custom op over the two halves
            half = W // 2
            u = treep.tile([128, half], F16, tag="mm_scratch")
            nc.vector._custom_dve(
                MINMIN, out=u[:], in0=d16[:, 0:half], in1=d16[:, half:W],
                s0=PAD_DIST, accum_out=R[:, t:t + 1],
            )
            # M block k is final after tile k: transpose+reduce finished
            # groups mid-loop so the serial tail shrinks to the last group.
            if (t + 1) % PER == 0:
                tail_group((t + 1) // PER - 1)

        # remaining groups
        for q in range(ROWT // PER, (NB + PER - 1) // PER):
            tail_group(q)

        nc.sync.dma_start(out1_d, R[:])
        nc.sync.dma_start(out2_d, C2[:])

    nc.compile()
    return nc


_CACHE = {}


def _get_program(n_rows, W):
    key = (n_rows, W)
    if key not in _CACHE:
        _CACHE[key] = build_program(n_rows, W)
    return _CACHE[key]


def run_device(xyz1, xyz2, trace=False):
    """Run the 8-core SPMD banded program; returns (dist1, dist2, res),
    dist1/dist2 in original (unsorted) order, exact after host patch."""
    xyz1 = np.asarray(xyz1)
    xyz2 = np.asarray(xyz2)
    B, N, _ = xyz1.shape
    M = xyz2.shape[1]
    W = BAND_W
    halves = N_CORES // B  # row-halves per batch (2)
    n_rows = N // halves
    C = n_rows + W
    nc = _get_program(n_rows, W)

    ident = np.eye(128, dtype=np.float16)

    # host prep: radius sort, augmented operands, per-core shifted rhs
    perms1, perms2 = [], []
    a_s, c_s, ra_s, rc_s = [], [], [], []
    rhs_aug = []
    for b in range(B):
        r1 = np.linalg.norm(xyz1[b].astype(np.float64), axis=1)
        r2 = np.linalg.norm(xyz2[b].astype(np.float64), axis=1)
        p1 = np.argsort(r1, kind="stable")
        p2 = np.argsort(r2, kind="stable")
        perms1.append(p1); perms2.append(p2)
        a_s.append(xyz1[b][p1]); c_s.append(xyz2[b][p2])
        ra_s.append(r1[p1]); rc_s.append(r2[p2])
        rhs_aug.append(_aug_rows(c_s[b], "rhs"))

    # padding column (far away): contributes 4*sh = PAD_DIST
    pad_col = np.zeros((K_AUG,), np.float16)
    pad_col[9] = np.float16(4.0)
    pad_col[10] = np.float16(4.0)
    pad_col[11] = np.float16(PAD_DIST / 4.0)
    pad_col[12] = np.float16(0.0)

    in_maps = []
    bases = []
    for c in range(N_CORES):
        b, h = divmod(c, halves)
        lhsT = _aug_rows(a_s[b][h * n_rows:(h + 1) * n_rows], "lhs")
        base = h * n_rows + 64 - W // 2  # global col of local col 0
        bases.append(base)
        rhs = np.repeat(pad_col[:, None], C, axis=1).astype(np.float16)
        g0, g1 = max(base, 0), min(base + C, M)
        rhs[:, g0 - base:g1 - base] = rhs_aug[b][:, g0:g1]
        in_maps.append({"lhsT": lhsT, "rhs": rhs, "ident": ident})

    res = run_bass_kernel_spmd(nc, in_maps, list(range(N_CORES)), trace=trace)

    ROWT = n_rows // 128
    COV = 128 * (ROWT - 1) + W
    NB = (COV + 127) // 128
    dist1_s = np.empty((B, N), np.float64)
    dist2_s = np.full((B, M), np.inf, np.float64)
    for c in range(N_CORES):
        b, h = divmod(c, halves)
        o1 = res.results[c]["out1"].astype(np.float64)  # [128, ROWT]
        o2 = res.results[c]["out2"].astype(np.float64)  # [128, NB]
        dist1_s[b, h * n_rows:(h + 1) * n_rows] = o1.T.reshape(-1)
        j_loc = np.arange(NB * 128)
        cols = bases[c] + j_loc
        valid = (cols >= 0) & (cols < M) & (j_loc < COV)
        np.minimum.at(dist2_s[b], cols[valid], o2.T.reshape(-1)[valid])
    dist1_s /= DSCALE
    dist2_s /= DSCALE

    # --- exact host patch for at-risk points -----------------------------
    # Window of global tile T (row range [128T,128T+128)): global cols
    # [64 - W/2 + 128T, 64 + W/2 + 128T) intersected with [0, M).
    nt = N // 128
    w_lo = np.maximum(64 - W // 2 + 128 * np.arange(nt), 0)
    w_hi = np.minimum(64 + W // 2 + 128 * np.arange(nt), M)
    for b in range(B):
        ra, rc = ra_s[b], rc_s[b]
        a, cc = a_s[b], c_s[b]
        # coverage radius per sorted row: window covers rc[w_lo[t]..w_hi[t]-1]
        cov1 = np.empty(N)
        for t in range(nt):
            lo = -np.inf if w_lo[t] == 0 else rc[w_lo[t]]
            hi = np.inf if w_hi[t] == M else rc[w_hi[t] - 1]
            rr = ra[128 * t:128 * (t + 1)]
            cov1[128 * t:128 * (t + 1)] = np.minimum(rr - lo, hi - rr)
        # coverage per sorted col: tiles t with w_lo[t] <= j < w_hi[t] form a
        # contiguous range; their rows span a contiguous sorted-row range.
        j_all = np.arange(M)
        ft = np.searchsorted(w_hi - 1, j_all, "left")   # first tile covering j
        lt = np.searchsorted(w_lo, j_all, "right") - 1  # last tile covering j
        lo_r = np.where(ft <= 0, -np.inf, ra[np.minimum(ft * 128, N - 1)])
        hi_r = np.where(lt >= nt - 1, np.inf, ra[np.minimum((lt + 1) * 128 - 1, N - 1)])
        cov2 = np.minimum(rc - lo_r, hi_r - rc)
        risk1 = np.where(dist1_s[b] > cov1 ** 2 * 0.997 - 1e-6)[0]
        risk2 = np.where(dist2_s[b] > cov2 ** 2 * 0.997 - 1e-6)[0]
        if len(risk1):
            d = ((a[risk1][:, None, :].astype(np.float64)
                  - cc[None, :, :].astype(np.float64)) ** 2).sum(-1)
            dist1_s[b][risk1] = d.min(1)
        if len(risk2):
            d = ((cc[risk2][:, None, :].astype(np.float64)
                  - a[None, :, :].astype(np.float64)) ** 2).sum(-1)
            dist2_s[b][risk2] = d.min(1)

    # unsort back to original order
    dist1 = np.empty_like(dist1_s)
    dist2 = np.empty_like(dist2_s)
    for b in range(B):
        dist1[b][perms1[b]] = dist1_s[b]
        dist2[b][perms2[b]] = dist2_s[b]
    return dist1, dist2, res


def _finalize(dist1, dist2):
    dist1 = np.maximum(dist1, 0.0)
    dist2 = np.maximum(dist2, 0.0)
    cd_p = (np.sqrt(dist1).mean(axis=1) + np.sqrt(dist2).mean(axis=1)) / 2.0
    cd_t = dist1.mean(axis=1) + dist2.mean(axis=1)
    p1 = (dist1 < F1_THRESHOLD).mean(axis=1)
    p2 = (dist2 < F1_THRESHOLD).mean(axis=1)
    denom = p1 + p2
    f1 = np.where(denom > 0, 2.0 * p1 * p2 / np.where(denom > 0, denom, 1.0), 0.0)
    return (cd_p.astype(np.float32), cd_t.astype(np.float32),
            f1.astype(np.float32))


def kernel(xyz1, xyz2):
    dist1, dist2, _ = run_device(xyz1, xyz2, trace=False)
    return _finalize(dist1, dist2)


# revision 13
# speedup vs baseline: 1.1264x; 1.1264x over previous
"""Chamfer distance + F1 kernel for Trainium2 (8 NeuronCores).

Banded-KNN strategy (B=4 batches, N=M=8192 points, 3D):
  - Host sorts each batch's clouds by radius r=||p||.  Since radius is
    1-Lipschitz (|r_x - r_y| <= ||x-y||), a point's nearest neighbour is
    radially close whenever it is close in space, so a narrow band around
    the radius-sorted diagonal contains the NN for all but a handful of
    isolated points.
  - core c handles batch b = c//2, sorted-row-half h = c%2 (4096 rows).
    Each 128-row tile t computes scaled squared distances only against a
    static W-wide window of radius-sorted xyz2 (window slides 128 cols per
    tile).  The host hands each core a pre-shifted, padded slice of sorted
    xyz2 so the compiled program is identical across cores (SPMD).
  - PE computes the W-wide block in fp16 via the augmented K=13 contraction
    (fp32 operands hi/lo-split into fp16), ACT converts PSUM fp32 -> SBUF
    fp16 with Relu, DVE does the banded row-min (fused custom MINMIN op at
    2x) and a sliding col-min accumulator M (fp16 tensor_tensor at 2x).
    PE transposes finished M blocks mid-loop, DVE reduces -> col mins.
  - Host patch: points whose banded min exceeds the window's radial
    coverage (provable criterion) get exact numpy rows (~35/batch-side),
    making the result exact up to fp16 rounding.  cd/f1 host finalize.
"""

import sys

if "/opt/trn_rl_repo" not in sys.path:
    sys.path.insert(0, "/opt/trn_rl_repo")

from contextlib import ExitStack

import numpy as np

import concourse.tile as tile
import concourse.dve_ops as dve_ops
from concourse import bacc, bass_isa, mybir
from concourse.bass_utils import run_bass_kernel_spmd
from concourse.dve_spec import C0, AluOp, Spec, Src0, Src1, lower, minn
from concourse.dve_uop import DveOpSpec

F16 = mybir.dt.float16
F32 = mybir.dt.float32
MIN = mybir.AluOpType.min
AXX = mybir.AxisListType.X

SCALE = 24.0  # coordinate prescale; distances come out scaled by SCALE**2
DSCALE = SCALE * SCALE
F1_THRESHOLD = 1e-4
PAD_DIST = 60000.0  # scaled distance of padding columns (> max real ~53k)

N_CORES = 8
K_AUG = 13  # 9 coord-product rows + 2 sq1 rows + 2 sq2 rows
BAND_W = 768  # band width (columns per row-tile window)
import os as _os
USE_MINMIN_2X = _os.environ.get("MINMIN_2X", "0") == "1"

_RealCDA = bass_isa.InstCustomDveAnt


def _cda_perf(*a, **kw):
    """bass.py constructs InstCustomDveAnt via module attr; force perf_max=1
    (2x_1PORT opt-in, instruction byte 36[7:6]) for the MINMIN op."""
    inst = _RealCDA(*a, **kw)
    if USE_MINMIN_2X and kw.get("op_name") == "MINMIN_REDUCE_ANT":
        inst.perf_max = 1
    return inst


bass_isa.InstCustomDveAnt = _cda_perf


def _split16(v):
    """Split fp32 array into fp16 hi + lo so hi+lo ~= v to ~2^-22 rel."""
    hi = v.astype(np.float16)
    lo = (v - hi.astype(np.float32)).astype(np.float16)
    return hi, lo


def _aug_rows(xyz, side):
    """Augmented fp16 operand rows [K_AUG, n] for one cloud.

    side='lhs': rows for xyz1 (stationary), side='rhs': rows for xyz2
    (moving), such that sum_k lhsT[k,p]*rhs[k,f] ~= DSCALE*||x1_p - x2_f||^2.
    """
    n = xyz.shape[0]
    out = np.empty((K_AUG, n), np.float16)
    if side == "lhs":
        v = (-2.0 * SCALE) * xyz.astype(np.float32)
        h, l = _split16(v)
        sq = ((SCALE * xyz.astype(np.float32)) ** 2).sum(axis=1) * 0.25
        sh, sl = _split16(sq)
        for c in range(3):
            out[3 * c + 0] = h[:, c]
            out[3 * c + 1] = h[:, c]
            out[3 * c + 2] = l[:, c]
        out[9] = sh
        out[10] = sl
        out[11] = np.float16(4.0)
        out[12] = np.float16(4.0)
    else:
        w = SCALE * xyz.astype(np.float32)
        h, l = _split16(w)
        sq = ((w.astype(np.float64) ** 2).sum(axis=1) * 0.25).astype(np.float32)
        sh, sl = _split16(sq)
        for c in range(3):
            out[3 * c + 0] = h[:, c]
            out[3 * c + 1] = l[:, c]
            out[3 * c + 2] = h[:, c]
        out[9] = np.float16(4.0)
        out[10] = np.float16(4.0)
        out[11] = sh
        out[12] = sl
    return out


def _build_minmin_2x(uops_1x):
    """Hand-authored 2x_1PORT uop program for MINMIN (no stock accum op runs
    above 1x; this processes 2 packed fp16 pairs per port per cycle).

    steady-state, per cycle:
      lanes: 1=SRC_0 2=SRC_1 3=SRC_0_HI 4=SRC_1_HI
      s0: lo = MIN(SRC_0, SRC_1); delay lanes carry the HI pair forward
      s1: hi = MIN(SRC_0_HI, SRC_1_HI); delay0 captures s0's lo (realign)
      s2: merged = MIN(hi[curr], lo[delay0])
      s3: acc = MIN(merged[curr], acc[self])  (alu_out_a = accumulator)
      s4-7: hold.  out tensor gets scratch values; only accum_out is used.
    """
    from copy import deepcopy

    from concourse.dve_uop import (
        AluInp, DelayInp, InpSel, OutPath, OutSel, Trigger,
        UopConfig, UopDpConfig,
    )
    from concourse.dve_uop import AluOp as UAluOp

    PD = DelayInp.PREV_DELAY
    PAO = DelayInp.PREV_ALU_OUT

    def dp(op, s0, s1, a=0):
        return UopDpConfig(
            op=op, alu_src0=s0, alu_src1=s1,
            delay=[PD, PD, PD, PD, PAO, PAO, PAO],
            alu_out_enable=1, swap_enable=0,
            alu_out_a_enable=a, alu_out_b_enable=0,
            delay_enable=[1, 0, 0, 0, 0, 0, 0],
            idx0_sel=0, idx1_sel=0,
        )

    s0 = dp(UAluOp.MIN, AluInp.PREV_DELAY_0, AluInp.PREV_DELAY_1)
    s0.delay_enable = [1, 1, 1, 1, 0, 0, 0]
    s1 = dp(UAluOp.MIN, AluInp.PREV_DELAY_2, AluInp.PREV_DELAY_3)
    s1.delay = [PAO, PD, PD, PD, PAO, PAO, PAO]
    stages = [
        s0,
        s1,
        # s2: PREV_ALU_OUT = s1's hi (prev block), PREV_DELAY_0 = lo
        dp(UAluOp.MIN, AluInp.PREV_ALU_OUT, AluInp.PREV_DELAY_0),
        # s3: CURR_ALU_OUT = own flop = the accumulator (seeded with C0)
        dp(UAluOp.MIN, AluInp.CURR_ALU_OUT, AluInp.PREV_ALU_OUT, a=1),
        dp(UAluOp.BYPASS, AluInp.PREV_ALU_OUT, AluInp.PREV_ALU_OUT, a=1),
        dp(UAluOp.BYPASS, AluInp.PREV_ALU_OUT, AluInp.PREV_ALU_OUT, a=1),
        dp(UAluOp.BYPASS, AluInp.PREV_ALU_OUT, AluInp.PREV_ALU_OUT, a=1),
        dp(UAluOp.BYPASS, AluInp.PREV_ALU_OUT, AluInp.PREV_ALU_OUT, a=1),
    ]

    def seed_dp():
        # seed token (1 cycle): C0 enters on input lane 3, rides delay
        # lane 2 to stage 3 which loads the accumulator flop; stage 0
        # bypasses C0 so stage 1's delay lane 0 (the merge operand in the
        # steady state) also starts at C0 instead of stale garbage.
        st0 = dp(UAluOp.BYPASS, AluInp.PREV_DELAY_2, AluInp.PREV_DELAY_2)
        st0.delay_enable = [1, 1, 1, 0, 0, 0, 0]
        st1 = dp(UAluOp.BYPASS, AluInp.PREV_ALU_OUT, AluInp.PREV_ALU_OUT)
        st1.delay = [PAO, PD, PD, PD, PAO, PAO, PAO]
        st1.delay_enable = [1, 0, 1, 0, 0, 0, 0]
        st2 = dp(UAluOp.BYPASS, AluInp.PREV_ALU_OUT, AluInp.PREV_ALU_OUT)
        st2.delay_enable = [1, 0, 1, 0, 0, 0, 0]
        st3 = dp(UAluOp.BYPASS, AluInp.PREV_DELAY_2, AluInp.PREV_DELAY_2, a=1)
        return [st0, st1, st2, st3] + [
            dp(UAluOp.BYPASS, AluInp.PREV_ALU_OUT, AluInp.PREV_ALU_OUT, a=1)
            for _ in range(4)
        ]
    steady = UopConfig(
        inp=[InpSel.ZERO, InpSel.SRC_0, InpSel.SRC_1, InpSel.SRC_0_HI,
             InpSel.SRC_1_HI, InpSel.ZERO, InpSel.ZERO, InpSel.ZERO],
        inp_enable=[0, 1, 1, 1, 1, 0, 0, 0],
        out={OutPath.WR0_LO: OutSel.ALU_OUT, OutPath.WR0_HI: OutSel.DELAY_0,
             OutPath.WR1_LO: OutSel.ALU_OUT, OutPath.WR1_HI: OutSel.ALU_OUT},
        out_enable={OutPath.WR0_LO: 1, OutPath.WR0_HI: 1,
                    OutPath.WR1_LO: 0, OutPath.WR1_HI: 0},
        out_last_subdim_enable=0,
        force_two_data_zero=0, force_two_data_one=0,
        require_inp0=1, require_inp1=1,
        repeat_count=0,
        trigger=(Trigger.SRC_TENSOR_DONE, Trigger.NONE, Trigger.NONE),
        next_uop=(0, 0, 0),
        inc_parameter_index=0, enable_rev_ops=0,
        match_mask=0, valid_match=0, replace_on_match=0, clear_match=0,
        write_predicate_select=0, write_predicate_enable=0,
        delay_shift8=0, index_increment=0, index_clear=0,
        accum_enabled=1, v4={},
        datapath_config=stages,
    )
    seed = deepcopy(uops_1x[0])
    seed.datapath_config = seed_dp()
    return [seed, steady]


def _register_minmin_reduce():
    """Custom DVE op: out = min(in0, in1); accum_out = min-fold(out)."""
    name = "MINMIN_REDUCE_ANT"
    if name in dve_ops._SUB_OPCODE_FOR_NAME:
        return next(op for op in dve_ops.OPS if op.name == name)

    def _ref(in0, in1, c0, c1, c2):
        out = np.minimum(np.asarray(in0, np.float32), np.asarray(in1, np.float32))
        acc = out.reshape(out.shape[0], -1).min(axis=-1, keepdims=True)
        acc = np.minimum(acc, c0)
        return out, acc

    spec = Spec(body=minn(Src0, Src1), accum=AluOp.MIN, accum_init=C0,
                reference=_ref)
    row = max(dve_ops._SUB_OPCODE_FOR_NAME.values()) + 1
    u1 = lower(spec, ver="v3")
    s3 = DveOpSpec(name=name, opcode=row, uops=u1,
                   uops_2x=_build_minmin_2x(u1) if USE_MINMIN_2X else None,
                   rd1_en=True, perf_max=1 if USE_MINMIN_2X else 0)
    s3.validate("v3")
    shas = {"v3": s3.sha("v3")}
    try:
        u1v4 = lower(spec, ver="v4")
        s4 = DveOpSpec(name=name, opcode=row, uops=u1v4, rd1_en=True)
        shas["v4"] = s4.sha("v4")
        dve_ops._COMPILE_CACHE[(name, "v4")] = s4
    except Exception:
        pass
    op = dve_ops.DveOp(name, spec, subdim=False, uops_sha=shas)
    dve_ops._COMPILE_CACHE[(name, "v3")] = s3
    dve_ops.OPS.append(op)
    dve_ops.CUSTOM_DVE_SPECS[name] = spec
    dve_ops._SUB_OPCODE_FOR_NAME[name] = row
    return op


def build_program(n_rows=4096, W=BAND_W):
    """Per-core banded program (SPMD-identical across cores).

    Local column space C = n_rows + W; tile t (128 rows) sees window
    [128*t, 128*t + W).  The host pre-shifts/pads each core's rhs so this
    static window pattern is centred on the tile's radius range.  The
    column-min output is the raw fp16 accumulator M, DMA'd out in chunks
    as its blocks finalize; the host does the 128-way partition min.
    """
    ROWT = n_rows // 128
    C = n_rows + W
    PER = 8  # tiles per M-chunk DMA-out
    PSW = 1024  # psum slot width per tile (bank aligned)
    NMM = (W + 511) // 512

    MINMIN = _register_minmin_reduce()
    nc = bacc.Bacc("TRN2", target_bir_lowering=False, debug=False,
                   num_devices=N_CORES)
    lhsT_d = nc.dram_tensor("lhsT", [K_AUG, n_rows], F16, kind="ExternalInput").ap()
    rhs_d = nc.dram_tensor("rhs", [K_AUG, C], F16, kind="ExternalInput").ap()
    out1_d = nc.dram_tensor("out1", [128, ROWT], F32, kind="ExternalOutput").ap()
    out2_d = nc.dram_tensor("out2", [128, C], F16, kind="ExternalOutput").ap()

    with tile.TileContext(nc) as tc, ExitStack() as ctx:
        const = ctx.enter_context(tc.tile_pool(name="const", bufs=1))
        d16p = ctx.enter_context(tc.tile_pool(name="d16", bufs=3))
        mp = ctx.enter_context(tc.tile_pool(name="macc", bufs=1))
        treep = ctx.enter_context(tc.tile_pool(name="tree", bufs=2))
        outp = ctx.enter_context(tc.tile_pool(name="outs", bufs=1))
        psp = ctx.enter_context(tc.tile_pool(name="ps", bufs=2, space="PSUM"))

        M = mp.tile([128, C], F16)
        R = outp.tile([128, ROWT], F32)

        # init col-min accumulator while input DMAs are in flight
        nc.vector.memset(M[:, 0:C // 2], PAD_DIST)
        nc.vector.memset(M[:, C // 2:C], PAD_DIST)

        # inputs: split across the two HWDGE queues (sync + scalar), in
        # first-needed-first order so the pipeline lights up early.
        w_sb = const.tile([K_AUG, n_rows], F16)
        r_sb = const.tile([K_AUG, C], F16)
        nc.sync.dma_start(r_sb[:, 0:W + 256], rhs_d[:, 0:W + 256])
        nc.scalar.dma_start(w_sb[:, 0:512], lhsT_d[:, 0:512])
        s = W + 256
        while s < C:
            w = min(1536, C - s)
            nc.sync.dma_start(r_sb[:, s:s + w], rhs_d[:, s:s + w])
            s += w
        s = 512
        while s < n_rows:
            w = min(1024, n_rows - s)
            nc.scalar.dma_start(w_sb[:, s:s + w], lhsT_d[:, s:s + w])
            s += w

        half = W // 2
        for tp in range(ROWT // 2):
            # pair of row tiles sharing one PSUM allocation + one ACTIVATE
            ps = psp.tile([128, 2 * PSW], F32, tag="ps")
            for i in range(2):
                t = 2 * tp + i
                for j in range(NMM):
                    wj = min(512, W - 512 * j)
                    nc.tensor.matmul(
                        ps[:, PSW * i + 512 * j:PSW * i + 512 * j + wj],
                        w_sb[:, 128 * t:128 * (t + 1)],
                        r_sb[:, 128 * t + 512 * j:128 * t + 512 * j + wj],
                        start=True, stop=True,
                    )
            d16 = d16p.tile([128, 2 * W], F16, tag="d16")
            nc.scalar.activation(
                d16[:].rearrange("p (i w) -> p i w", i=2),
                ps[:].rearrange("p (i w) -> p i w", i=2)[:, :, 0:W],
                mybir.ActivationFunctionType.Relu,
            )
            for i in range(2):
                t = 2 * tp + i
                lo = 128 * t
                dt = d16[:, W * i:W * (i + 1)]
                nc.vector.tensor_tensor(M[:, lo:lo + W], M[:, lo:lo + W],
                                        dt, op=MIN)
                u = treep.tile([128, half], F16, tag="mm_scratch")
                nc.vector._custom_dve(
                    MINMIN, out=u[:], in0=dt[:, 0:half], in1=dt[:, half:W],
                    s0=PAD_DIST, accum_out=R[:, t:t + 1],
                )
            # M cols [0, 128*t+128) are final after tile t: stream finished
            # chunks to DRAM during the loop (host does the partition-min).
            t = 2 * tp + 1
            if (t + 1) % PER == 0:
                q = (t + 1) // PER - 1
                c0, c1 = q * PER * 128, (q + 1) * PER * 128
                nc.sync.dma_start(out2_d[:, c0:c1], M[:, c0:c1])

        c0 = (ROWT // PER) * PER * 128
        nc.sync.dma_start(out2_d[:, c0:C], M[:, c0:C])
        nc.sync.dma_start(out1_d, R[:])

    nc.compile()
    return nc


_CACHE = {}


def _get_program(n_rows, W):
    key = (n_rows, W)
    if key not in _CACHE:
        _CACHE[key] = build_program(n_rows, W)
    return _CACHE[key]


def run_device(xyz1, xyz2, trace=False):
    """Run the 8-core SPMD banded program; returns (dist1, dist2, res),
    dist1/dist2 in original (unsorted) order, exact after host patch."""
    xyz1 = np.asarray(xyz1)
    xyz2 = np.asarray(xyz2)
    B, N, _ = xyz1.shape
    M = xyz2.shape[1]
    W = BAND_W
    halves = N_CORES // B  # row-halves per batch (2)
    n_rows = N // halves
    C = n_rows + W
    nc = _get_program(n_rows, W)

    # host prep: radius sort, augmented operands, per-core shifted rhs
    perms1, perms2 = [], []
    a_s, c_s, ra_s, rc_s = [], [], [], []
    rhs_aug = []
    for b in range(B):
        r1 = np.linalg.norm(xyz1[b].astype(np.float64), axis=1)
        r2 = np.linalg.norm(xyz2[b].astype(np.float64), axis=1)
        p1 = np.argsort(r1, kind="stable")
        p2 = np.argsort(r2, kind="stable")
        perms1.append(p1); perms2.append(p2)
        a_s.append(xyz1[b][p1]); c_s.append(xyz2[b][p2])
        ra_s.append(r1[p1]); rc_s.append(r2[p2])
        rhs_aug.append(_aug_rows(c_s[b], "rhs"))

    # padding column (far away): contributes 4*sh = PAD_DIST
    pad_col = np.zeros((K_AUG,), np.float16)
    pad_col[9] = np.float16(4.0)
    pad_col[10] = np.float16(4.0)
    pad_col[11] = np.float16(PAD_DIST / 4.0)
    pad_col[12] = np.float16(0.0)

    in_maps = []
    bases = []
    for c in range(N_CORES):
        b, h = divmod(c, halves)
        lhsT = _aug_rows(a_s[b][h * n_rows:(h + 1) * n_rows], "lhs")
        base = h * n_rows + 64 - W // 2  # global col of local col 0
        bases.append(base)
        rhs = np.repeat(pad_col[:, None], C, axis=1).astype(np.float16)
        g0, g1 = max(base, 0), min(base + C, M)
        rhs[:, g0 - base:g1 - base] = rhs_aug[b][:, g0:g1]
        in_maps.append({"lhsT": lhsT, "rhs": rhs})

    res = run_bass_kernel_spmd(nc, in_maps, list(range(N_CORES)), trace=trace)

    ROWT = n_rows // 128
    COV = 128 * (ROWT - 1) + W
    dist1_s = np.empty((B, N), np.float64)
    dist2_s = np.full((B, M), np.inf, np.float64)
    for c in range(N_CORES):
        b, h = divmod(c, halves)
        o1 = res.results[c]["out1"].astype(np.float64)  # [128, ROWT]
        o2 = res.results[c]["out2"]  # [128, C] fp16 raw col-min accumulator
        dist1_s[b, h * n_rows:(h + 1) * n_rows] = o1.T.reshape(-1)
        colmin = o2.astype(np.float32).min(axis=0).astype(np.float64)
        j_loc = np.arange(C)
        cols = bases[c] + j_loc
        valid = (cols >= 0) & (cols < M) & (j_loc < COV)
        np.minimum.at(dist2_s[b], cols[valid], colmin[valid])
    dist1_s /= DSCALE
    dist2_s /= DSCALE

    # --- exact host patch for at-risk points -----------------------------
    # Window of global tile T (row range [128T,128T+128)): global cols
    # [64 - W/2 + 128T, 64 + W/2 + 128T) intersected with [0, M).
    nt = N // 128
    w_lo = np.maximum(64 - W // 2 + 128 * np.arange(nt), 0)
    w_hi = np.minimum(64 + W // 2 + 128 * np.arange(nt), M)
    for b in range(B):
        ra, rc = ra_s[b], rc_s[b]
        a, cc = a_s[b], c_s[b]
        # coverage radius per sorted row: window covers rc[w_lo[t]..w_hi[t]-1]
        cov1 = np.empty(N)
        for t in range(nt):
            lo = -np.inf if w_lo[t] == 0 else rc[w_lo[t]]
            hi = np.inf if w_hi[t] == M else rc[w_hi[t] - 1]
            rr = ra[128 * t:128 * (t + 1)]
            cov1[128 * t:128 * (t + 1)] = np.minimum(rr - lo, hi - rr)
        # coverage per sorted col: tiles t with w_lo[t] <= j < w_hi[t] form a
        # contiguous range; their rows span a contiguous sorted-row range.
        j_all = np.arange(M)
        ft = np.searchsorted(w_hi - 1, j_all, "left")   # first tile covering j
        lt = np.searchsorted(w_lo, j_all, "right") - 1  # last tile covering j
        lo_r = np.where(ft <= 0, -np.inf, ra[np.minimum(ft * 128, N - 1)])
        hi_r = np.where(lt >= nt - 1, np.inf, ra[np.minimum((lt + 1) * 128 - 1, N - 1)])
        cov2 = np.minimum(rc - lo_r, hi_r - rc)
        risk1 = np.where(dist1_s[b] > cov1 ** 2 * 0.997 - 1e-6)[0]
        risk2 = np.where(dist2_s[b] > cov2 ** 2 * 0.997 - 1e-6)[0]
        if len(risk1):
            d = ((a[risk1][:, None, :].astype(np.float64)
                  - cc[None, :, :].astype(np.float64)) ** 2).sum(-1)
            dist1_s[b][risk1] = d.min(1)
        if len(risk2):
            d = ((cc[risk2][:, None, :].astype(np.float64)
                  - a[None, :, :].astype(np.float64)) ** 2).sum(-1)
            dist2_s[b][risk2] = d.min(1)

    # unsort back to original order
    dist1 = np.empty_like(dist1_s)
    dist2 = np.empty_like(dist2_s)
    for b in range(B):
        dist1[b][perms1[b]] = dist1_s[b]
        dist2[b][perms2[b]] = dist2_s[b]
    return dist1, dist2, res


def _finalize(dist1, dist2):
    dist1 = np.maximum(dist1, 0.0)
    dist2 = np.maximum(dist2, 0.0)
    cd_p = (np.sqrt(dist1).mean(axis=1) + np.sqrt(dist2).mean(axis=1)) / 2.0
    cd_t = dist1.mean(axis=1) + dist2.mean(axis=1)
    p1 = (dist1 < F1_THRESHOLD).mean(axis=1)
    p2 = (dist2 < F1_THRESHOLD).mean(axis=1)
    denom = p1 + p2
    f1 = np.where(denom > 0, 2.0 * p1 * p2 / np.where(denom > 0, denom, 1.0), 0.0)
    return (cd_p.astype(np.float32), cd_t.astype(np.float32),
            f1.astype(np.float32))


def kernel(xyz1, xyz2):
    dist1, dist2, _ = run_device(xyz1, xyz2, trace=False)
    return _finalize(dist1, dist2)


# revision 15
# speedup vs baseline: 1.6126x; 1.4316x over previous
"""Chamfer distance + F1 kernel for Trainium2 (8 NeuronCores).

Banded-KNN strategy (B=4 batches, N=M=8192 points, 3D):
  - Host sorts each batch's clouds by radius r=||p||.  Since radius is
    1-Lipschitz (|r_x - r_y| <= ||x-y||), a point's nearest neighbour is
    radially close whenever it is close in space, so a narrow band around
    the radius-sorted diagonal contains the NN for all but a handful of
    isolated points.
  - core c handles batch b = c//2, sorted-row-half h = c%2 (4096 rows).
    Each 128-row tile t computes scaled squared distances only against a
    static W-wide window of radius-sorted xyz2 (window slides 128 cols per
    tile).  The host hands each core a pre-shifted, padded slice of sorted
    xyz2 so the compiled program is identical across cores (SPMD).
  - PE computes the W-wide block in fp16 via the augmented K=13 contraction
    (fp32 operands hi/lo-split into fp16), ACT converts PSUM fp32 -> SBUF
    fp16 with Relu, DVE does the banded row-min (fused custom MINMIN op at
    2x) and a sliding col-min accumulator M (fp16 tensor_tensor at 2x).
    PE transposes finished M blocks mid-loop, DVE reduces -> col mins.
  - Host patch: points whose banded min exceeds the window's radial
    coverage (provable criterion) get exact numpy rows (~35/batch-side),
    making the result exact up to fp16 rounding.  cd/f1 host finalize.
"""

import sys

if "/opt/trn_rl_repo" not in sys.path:
    sys.path.insert(0, "/opt/trn_rl_repo")

from contextlib import ExitStack

import numpy as np

import concourse.tile as tile
import concourse.dve_ops as dve_ops
from concourse import bacc, bass_isa, mybir
from concourse.bass_utils import run_bass_kernel_spmd
from concourse.dve_spec import C0, AluOp, Spec, Src0, Src1, lower, minn
from concourse.dve_uop import DveOpSpec

F16 = mybir.dt.float16
F32 = mybir.dt.float32
MIN = mybir.AluOpType.min
AXX = mybir.AxisListType.X

SCALE = 24.0  # coordinate prescale; distances come out scaled by SCALE**2
DSCALE = SCALE * SCALE
F1_THRESHOLD = 1e-4
PAD_DIST = 60000.0  # scaled distance of padding columns (> max real ~53k)

N_CORES = 8
K_AUG = 13  # 9 coord-product rows + 2 sq1 rows + 2 sq2 rows
BAND_W = 512  # band width (columns per row-tile window)
import os as _os
USE_MINMIN_2X = _os.environ.get("MINMIN_2X", "0") == "1"

_RealCDA = bass_isa.InstCustomDveAnt


def _cda_perf(*a, **kw):
    """bass.py constructs InstCustomDveAnt via module attr; force perf_max=1
    (2x_1PORT opt-in, instruction byte 36[7:6]) for the MINMIN op."""
    inst = _RealCDA(*a, **kw)
    if USE_MINMIN_2X and kw.get("op_name") == "MINMIN_REDUCE_ANT":
        inst.perf_max = 1
    return inst


bass_isa.InstCustomDveAnt = _cda_perf


def _split16(v):
    """Split fp32 array into fp16 hi + lo so hi+lo ~= v to ~2^-22 rel."""
    hi = v.astype(np.float16)
    lo = (v - hi.astype(np.float32)).astype(np.float16)
    return hi, lo


def _aug_rows(xyz, side):
    """Augmented fp16 operand rows [K_AUG, n] for one cloud.

    side='lhs': rows for xyz1 (stationary), side='rhs': rows for xyz2
    (moving), such that sum_k lhsT[k,p]*rhs[k,f] ~= DSCALE*||x1_p - x2_f||^2.
    """
    n = xyz.shape[0]
    out = np.empty((K_AUG, n), np.float16)
    if side == "lhs":
        v = (-2.0 * SCALE) * xyz.astype(np.float32)
        h, l = _split16(v)
        sq = ((SCALE * xyz.astype(np.float32)) ** 2).sum(axis=1) * 0.25
        sh, sl = _split16(sq)
        for c in range(3):
            out[3 * c + 0] = h[:, c]
            out[3 * c + 1] = h[:, c]
            out[3 * c + 2] = l[:, c]
        out[9] = sh
        out[10] = sl
        out[11] = np.float16(4.0)
        out[12] = np.float16(4.0)
    else:
        w = SCALE * xyz.astype(np.float32)
        h, l = _split16(w)
        sq = ((w.astype(np.float64) ** 2).sum(axis=1) * 0.25).astype(np.float32)
        sh, sl = _split16(sq)
        for c in range(3):
            out[3 * c + 0] = h[:, c]
            out[3 * c + 1] = l[:, c]
            out[3 * c + 2] = h[:, c]
        out[9] = np.float16(4.0)
        out[10] = np.float16(4.0)
        out[11] = sh
        out[12] = sl
    return out


def _build_minmin_2x(uops_1x):
    """Hand-authored 2x_1PORT uop program for MINMIN (no stock accum op runs
    above 1x; this processes 2 packed fp16 pairs per port per cycle).

    steady-state, per cycle:
      lanes: 1=SRC_0 2=SRC_1 3=SRC_0_HI 4=SRC_1_HI
      s0: lo = MIN(SRC_0, SRC_1); delay lanes carry the HI pair forward
      s1: hi = MIN(SRC_0_HI, SRC_1_HI); delay0 captures s0's lo (realign)
      s2: merged = MIN(hi[curr], lo[delay0])
      s3: acc = MIN(merged[curr], acc[self])  (alu_out_a = accumulator)
      s4-7: hold.  out tensor gets scratch values; only accum_out is used.
    """
    from copy import deepcopy

    from concourse.dve_uop import (
        AluInp, DelayInp, InpSel, OutPath, OutSel, Trigger,
        UopConfig, UopDpConfig,
    )
    from concourse.dve_uop import AluOp as UAluOp

    PD = DelayInp.PREV_DELAY
    PAO = DelayInp.PREV_ALU_OUT

    def dp(op, s0, s1, a=0):
        return UopDpConfig(
            op=op, alu_src0=s0, alu_src1=s1,
            delay=[PD, PD, PD, PD, PAO, PAO, PAO],
            alu_out_enable=1, swap_enable=0,
            alu_out_a_enable=a, alu_out_b_enable=0,
            delay_enable=[1, 0, 0, 0, 0, 0, 0],
            idx0_sel=0, idx1_sel=0,
        )

    s0 = dp(UAluOp.MIN, AluInp.PREV_DELAY_0, AluInp.PREV_DELAY_1)
    s0.delay_enable = [1, 1, 1, 1, 0, 0, 0]
    s1 = dp(UAluOp.MIN, AluInp.PREV_DELAY_2, AluInp.PREV_DELAY_3)
    s1.delay = [PAO, PD, PD, PD, PAO, PAO, PAO]
    stages = [
        s0,
        s1,
        # s2: PREV_ALU_OUT = s1's hi (prev block), PREV_DELAY_0 = lo
        dp(UAluOp.MIN, AluInp.PREV_ALU_OUT, AluInp.PREV_DELAY_0),
        # s3: CURR_ALU_OUT = own flop = the accumulator (seeded with C0)
        dp(UAluOp.MIN, AluInp.CURR_ALU_OUT, AluInp.PREV_ALU_OUT, a=1),
        dp(UAluOp.BYPASS, AluInp.PREV_ALU_OUT, AluInp.PREV_ALU_OUT, a=1),
        dp(UAluOp.BYPASS, AluInp.PREV_ALU_OUT, AluInp.PREV_ALU_OUT, a=1),
        dp(UAluOp.BYPASS, AluInp.PREV_ALU_OUT, AluInp.PREV_ALU_OUT, a=1),
        dp(UAluOp.BYPASS, AluInp.PREV_ALU_OUT, AluInp.PREV_ALU_OUT, a=1),
    ]

    def seed_dp():
        # seed token (1 cycle): C0 enters on input lane 3, rides delay
        # lane 2 to stage 3 which loads the accumulator flop; stage 0
        # bypasses C0 so stage 1's delay lane 0 (the merge operand in the
        # steady state) also starts at C0 instead of stale garbage.
        st0 = dp(UAluOp.BYPASS, AluInp.PREV_DELAY_2, AluInp.PREV_DELAY_2)
        st0.delay_enable = [1, 1, 1, 0, 0, 0, 0]
        st1 = dp(UAluOp.BYPASS, AluInp.PREV_ALU_OUT, AluInp.PREV_ALU_OUT)
        st1.delay = [PAO, PD, PD, PD, PAO, PAO, PAO]
        st1.delay_enable = [1, 0, 1, 0, 0, 0, 0]
        st2 = dp(UAluOp.BYPASS, AluInp.PREV_ALU_OUT, AluInp.PREV_ALU_OUT)
        st2.delay_enable = [1, 0, 1, 0, 0, 0, 0]
        st3 = dp(UAluOp.BYPASS, AluInp.PREV_DELAY_2, AluInp.PREV_DELAY_2, a=1)
        return [st0, st1, st2, st3] + [
            dp(UAluOp.BYPASS, AluInp.PREV_ALU_OUT, AluInp.PREV_ALU_OUT, a=1)
            for _ in range(4)
        ]
    steady = UopConfig(
        inp=[InpSel.ZERO, InpSel.SRC_0, InpSel.SRC_1, InpSel.SRC_0_HI,
             InpSel.SRC_1_HI, InpSel.ZERO, InpSel.ZERO, InpSel.ZERO],
        inp_enable=[0, 1, 1, 1, 1, 0, 0, 0],
        out={OutPath.WR0_LO: OutSel.ALU_OUT, OutPath.WR0_HI: OutSel.DELAY_0,
             OutPath.WR1_LO: OutSel.ALU_OUT, OutPath.WR1_HI: OutSel.ALU_OUT},
        out_enable={OutPath.WR0_LO: 1, OutPath.WR0_HI: 1,
                    OutPath.WR1_LO: 0, OutPath.WR1_HI: 0},
        out_last_subdim_enable=0,
        force_two_data_zero=0, force_two_data_one=0,
        require_inp0=1, require_inp1=1,
        repeat_count=0,
        trigger=(Trigger.SRC_TENSOR_DONE, Trigger.NONE, Trigger.NONE),
        next_uop=(0, 0, 0),
        inc_parameter_index=0, enable_rev_ops=0,
        match_mask=0, valid_match=0, replace_on_match=0, clear_match=0,
        write_predicate_select=0, write_predicate_enable=0,
        delay_shift8=0, index_increment=0, index_clear=0,
        accum_enabled=1, v4={},
        datapath_config=stages,
    )
    seed = deepcopy(uops_1x[0])
    seed.datapath_config = seed_dp()
    return [seed, steady]


def _register_minmin_reduce():
    """Custom DVE op: out = min(in0, in1); accum_out = min-fold(out)."""
    name = "MINMIN_REDUCE_ANT"
    if name in dve_ops._SUB_OPCODE_FOR_NAME:
        return next(op for op in dve_ops.OPS if op.name == name)

    def _ref(in0, in1, c0, c1, c2):
        out = np.minimum(np.asarray(in0, np.float32), np.asarray(in1, np.float32))
        acc = out.reshape(out.shape[0], -1).min(axis=-1, keepdims=True)
        acc = np.minimum(acc, c0)
        return out, acc

    spec = Spec(body=minn(Src0, Src1), accum=AluOp.MIN, accum_init=C0,
                reference=_ref)
    row = max(dve_ops._SUB_OPCODE_FOR_NAME.values()) + 1
    u1 = lower(spec, ver="v3")
    s3 = DveOpSpec(name=name, opcode=row, uops=u1,
                   uops_2x=_build_minmin_2x(u1) if USE_MINMIN_2X else None,
                   rd1_en=True, perf_max=1 if USE_MINMIN_2X else 0)
    s3.validate("v3")
    shas = {"v3": s3.sha("v3")}
    try:
        u1v4 = lower(spec, ver="v4")
        s4 = DveOpSpec(name=name, opcode=row, uops=u1v4, rd1_en=True)
        shas["v4"] = s4.sha("v4")
        dve_ops._COMPILE_CACHE[(name, "v4")] = s4
    except Exception:
        pass
    op = dve_ops.DveOp(name, spec, subdim=False, uops_sha=shas)
    dve_ops._COMPILE_CACHE[(name, "v3")] = s3
    dve_ops.OPS.append(op)
    dve_ops.CUSTOM_DVE_SPECS[name] = spec
    dve_ops._SUB_OPCODE_FOR_NAME[name] = row
    return op


def build_program(n_rows=4096, W=BAND_W):
    """Per-core banded program (SPMD-identical across cores).

    Local column space C = n_rows + W; tile t (128 rows) sees window
    [128*t, 128*t + W).  The host pre-shifts/pads each core's rhs so this
    static window pattern is centred on the tile's radius range.  The
    column-min output is the raw fp16 accumulator M, DMA'd out in chunks
    as its blocks finalize; the host does the 128-way partition min.
    """
    ROWT = n_rows // 128
    C = n_rows + W
    PER = 8  # tiles per M-chunk DMA-out
    NMM = (W + 511) // 512
    PSW = 512 * NMM  # psum slot width per tile (bank aligned)

    MINMIN = _register_minmin_reduce()
    nc = bacc.Bacc("TRN2", target_bir_lowering=False, debug=False,
                   num_devices=N_CORES)
    lhsT_d = nc.dram_tensor("lhsT", [K_AUG, n_rows], F16, kind="ExternalInput").ap()
    rhs_d = nc.dram_tensor("rhs", [K_AUG, C], F16, kind="ExternalInput").ap()
    out1_d = nc.dram_tensor("out1", [128, ROWT], F32, kind="ExternalOutput").ap()
    out2_d = nc.dram_tensor("out2", [128, C], F16, kind="ExternalOutput").ap()

    with tile.TileContext(nc) as tc, ExitStack() as ctx:
        const = ctx.enter_context(tc.tile_pool(name="const", bufs=1))
        d16p = ctx.enter_context(tc.tile_pool(name="d16", bufs=3))
        mp = ctx.enter_context(tc.tile_pool(name="macc", bufs=1))
        treep = ctx.enter_context(tc.tile_pool(name="tree", bufs=2))
        outp = ctx.enter_context(tc.tile_pool(name="outs", bufs=1))
        psp = ctx.enter_context(tc.tile_pool(name="ps", bufs=2, space="PSUM"))

        M = mp.tile([128, C], F16)
        R = outp.tile([128, ROWT], F32)

        # init col-min accumulator while input DMAs are in flight
        nc.vector.memset(M[:, 0:C // 2], PAD_DIST)
        nc.vector.memset(M[:, C // 2:C], PAD_DIST)

        # inputs: split across the two HWDGE queues (sync + scalar), in
        # first-needed-first order so the pipeline lights up early.
        w_sb = const.tile([K_AUG, n_rows], F16)
        r_sb = const.tile([K_AUG, C], F16)
        nc.sync.dma_start(r_sb[:, 0:W + 256], rhs_d[:, 0:W + 256])
        nc.scalar.dma_start(w_sb[:, 0:512], lhsT_d[:, 0:512])
        s = W + 256
        while s < C:
            w = min(1536, C - s)
            nc.sync.dma_start(r_sb[:, s:s + w], rhs_d[:, s:s + w])
            s += w
        s = 512
        while s < n_rows:
            w = min(1024, n_rows - s)
            nc.scalar.dma_start(w_sb[:, s:s + w], lhsT_d[:, s:s + w])
            s += w

        half = W // 2
        for tp in range(ROWT // 2):
            # pair of row tiles sharing one PSUM allocation + one ACTIVATE
            ps = psp.tile([128, 2 * PSW], F32, tag="ps")
            for i in range(2):
                t = 2 * tp + i
                for j in range(NMM):
                    wj = min(512, W - 512 * j)
                    nc.tensor.matmul(
                        ps[:, PSW * i + 512 * j:PSW * i + 512 * j + wj],
                        w_sb[:, 128 * t:128 * (t + 1)],
                        r_sb[:, 128 * t + 512 * j:128 * t + 512 * j + wj],
                        start=True, stop=True,
                    )
            d16 = d16p.tile([128, 2 * W], F16, tag="d16")
            nc.scalar.activation(
                d16[:].rearrange("p (i w) -> p i w", i=2),
                ps[:].rearrange("p (i w) -> p i w", i=2)[:, :, 0:W],
                mybir.ActivationFunctionType.Relu,
            )
            for i in range(2):
                t = 2 * tp + i
                lo = 128 * t
                dt = d16[:, W * i:W * (i + 1)]
                nc.vector.tensor_tensor(M[:, lo:lo + W], M[:, lo:lo + W],
                                        dt, op=MIN)
                u = treep.tile([128, half], F16, tag="mm_scratch")
                nc.vector._custom_dve(
                    MINMIN, out=u[:], in0=dt[:, 0:half], in1=dt[:, half:W],
                    s0=PAD_DIST, accum_out=R[:, t:t + 1],
                )
            # M cols [0, 128*t+128) are final after tile t: stream finished
            # chunks to DRAM during the loop (host does the partition-min).
            t = 2 * tp + 1
            if (t + 1) % PER == 0:
                q = (t + 1) // PER - 1
                c0, c1 = q * PER * 128, (q + 1) * PER * 128
                nc.sync.dma_start(out2_d[:, c0:c1], M[:, c0:c1])

        c0 = (ROWT // PER) * PER * 128
        nc.sync.dma_start(out2_d[:, c0:C], M[:, c0:C])
        nc.sync.dma_start(out1_d, R[:])

    nc.compile()
    return nc


_CACHE = {}


def _get_program(n_rows, W):
    key = (n_rows, W)
    if key not in _CACHE:
        _CACHE[key] = build_program(n_rows, W)
    return _CACHE[key]


def run_device(xyz1, xyz2, trace=False):
    """Run the 8-core SPMD banded program; returns (dist1, dist2, res),
    dist1/dist2 in original (unsorted) order, exact after host patch."""
    xyz1 = np.asarray(xyz1)
    xyz2 = np.asarray(xyz2)
    B, N, _ = xyz1.shape
    M = xyz2.shape[1]
    W = BAND_W
    halves = N_CORES // B  # row-halves per batch (2)
    n_rows = N // halves
    C = n_rows + W
    nc = _get_program(n_rows, W)

    # host prep: radius sort, augmented operands, per-core shifted rhs
    perms1, perms2 = [], []
    a_s, c_s, ra_s, rc_s = [], [], [], []
    rhs_aug = []
    for b in range(B):
        r1 = np.linalg.norm(xyz1[b].astype(np.float64), axis=1)
        r2 = np.linalg.norm(xyz2[b].astype(np.float64), axis=1)
        p1 = np.argsort(r1, kind="stable")
        p2 = np.argsort(r2, kind="stable")
        perms1.append(p1); perms2.append(p2)
        a_s.append(xyz1[b][p1]); c_s.append(xyz2[b][p2])
        ra_s.append(r1[p1]); rc_s.append(r2[p2])
        rhs_aug.append(_aug_rows(c_s[b], "rhs"))

    # padding column (far away): contributes 4*sh = PAD_DIST
    pad_col = np.zeros((K_AUG,), np.float16)
    pad_col[9] = np.float16(4.0)
    pad_col[10] = np.float16(4.0)
    pad_col[11] = np.float16(PAD_DIST / 4.0)
    pad_col[12] = np.float16(0.0)

    in_maps = []
    bases = []
    for c in range(N_CORES):
        b, h = divmod(c, halves)
        lhsT = _aug_rows(a_s[b][h * n_rows:(h + 1) * n_rows], "lhs")
        base = h * n_rows + 64 - W // 2  # global col of local col 0
        bases.append(base)
        rhs = np.repeat(pad_col[:, None], C, axis=1).astype(np.float16)
        g0, g1 = max(base, 0), min(base + C, M)
        rhs[:, g0 - base:g1 - base] = rhs_aug[b][:, g0:g1]
        in_maps.append({"lhsT": lhsT, "rhs": rhs})

    res = run_bass_kernel_spmd(nc, in_maps, list(range(N_CORES)), trace=trace)

    ROWT = n_rows // 128
    COV = 128 * (ROWT - 1) + W
    dist1_s = np.empty((B, N), np.float64)
    dist2_s = np.full((B, M), np.inf, np.float64)
    for c in range(N_CORES):
        b, h = divmod(c, halves)
        o1 = res.results[c]["out1"].astype(np.float64)  # [128, ROWT]
        o2 = res.results[c]["out2"]  # [128, C] fp16 raw col-min accumulator
        dist1_s[b, h * n_rows:(h + 1) * n_rows] = o1.T.reshape(-1)
        colmin = o2.astype(np.float32).min(axis=0).astype(np.float64)
        j_loc = np.arange(C)
        cols = bases[c] + j_loc
        valid = (cols >= 0) & (cols < M) & (j_loc < COV)
        np.minimum.at(dist2_s[b], cols[valid], colmin[valid])
    dist1_s /= DSCALE
    dist2_s /= DSCALE

    # --- exact host patch for at-risk points -----------------------------
    # Window of global tile T (row range [128T,128T+128)): global cols
    # [64 - W/2 + 128T, 64 + W/2 + 128T) intersected with [0, M).
    nt = N // 128
    w_lo = np.maximum(64 - W // 2 + 128 * np.arange(nt), 0)
    w_hi = np.minimum(64 + W // 2 + 128 * np.arange(nt), M)
    for b in range(B):
        ra, rc = ra_s[b], rc_s[b]
        a, cc = a_s[b], c_s[b]
        # coverage radius per sorted row: window covers rc[w_lo[t]..w_hi[t]-1]
        cov1 = np.empty(N)
        for t in range(nt):
            lo = -np.inf if w_lo[t] == 0 else rc[w_lo[t]]
            hi = np.inf if w_hi[t] == M else rc[w_hi[t] - 1]
            rr = ra[128 * t:128 * (t + 1)]
            cov1[128 * t:128 * (t + 1)] = np.minimum(rr - lo, hi - rr)
        # coverage per sorted col: tiles t with w_lo[t] <= j < w_hi[t] form a
        # contiguous range; their rows span a contiguous sorted-row range.
        j_all = np.arange(M)
        ft = np.searchsorted(w_hi - 1, j_all, "left")   # first tile covering j
        lt = np.searchsorted(w_lo, j_all, "right") - 1  # last tile covering j
        lo_r = np.where(ft <= 0, -np.inf, ra[np.minimum(ft * 128, N - 1)])
        hi_r = np.where(lt >= nt - 1, np.inf, ra[np.minimum((lt + 1) * 128 - 1, N - 1)])
        cov2 = np.minimum(rc - lo_r, hi_r - rc)
        risk1 = np.where(dist1_s[b] > cov1 ** 2 * 0.997 - 1e-6)[0]
        risk2 = np.where(dist2_s[b] > cov2 ** 2 * 0.997 - 1e-6)[0]
        if len(risk1):
            d = ((a[risk1][:, None, :].astype(np.float64)
                  - cc[None, :, :].astype(np.float64)) ** 2).sum(-1)
            dist1_s[b][risk1] = d.min(1)
        if len(risk2):
            d = ((cc[risk2][:, None, :].astype(np.float64)
                  - a[None, :, :].astype(np.float64)) ** 2).sum(-1)
            dist2_s[b][risk2] = d.min(1)

    # unsort back to original order
    dist1 = np.empty_like(dist1_s)
    dist2 = np.empty_like(dist2_s)
    for b in range(B):
        dist1[b][perms1[b]] = dist1_s[b]
        dist2[b][perms2[b]] = dist2_s[b]
    return dist1, dist2, res


def _finalize(dist1, dist2):
    dist1 = np.maximum(dist1, 0.0)
    dist2 = np.maximum(dist2, 0.0)
    cd_p = (np.sqrt(dist1).mean(axis=1) + np.sqrt(dist2).mean(axis=1)) / 2.0
    cd_t = dist1.mean(axis=1) + dist2.mean(axis=1)
    p1 = (dist1 < F1_THRESHOLD).mean(axis=1)
    p2 = (dist2 < F1_THRESHOLD).mean(axis=1)
    denom = p1 + p2
    f1 = np.where(denom > 0, 2.0 * p1 * p2 / np.where(denom > 0, denom, 1.0), 0.0)
    return (cd_p.astype(np.float32), cd_t.astype(np.float32),
            f1.astype(np.float32))


def kernel(xyz1, xyz2):
    dist1, dist2, _ = run_device(xyz1, xyz2, trace=False)
    return _finalize(dist1, dist2)


# revision 17
# speedup vs baseline: 1.6757x; 1.0392x over previous
"""Chamfer distance + F1 kernel for Trainium2 (8 NeuronCores).

Banded-KNN strategy (B=4 batches, N=M=8192 points, 3D):
  - Host sorts each batch's clouds by radius r=||p||.  Since radius is
    1-Lipschitz (|r_x - r_y| <= ||x-y||), a point's nearest neighbour is
    radially close whenever it is close in space, so a narrow band around
    the radius-sorted diagonal contains the NN for all but a handful of
    isolated points.
  - core c handles batch b = c//2, sorted-row-half h = c%2 (4096 rows).
    Each 128-row tile t computes scaled squared distances only against a
    static W-wide window of radius-sorted xyz2 (window slides 128 cols per
    tile).  The host hands each core a pre-shifted, padded slice of sorted
    xyz2 so the compiled program is identical across cores (SPMD).
  - PE computes the W-wide block in fp16 via the augmented K=13 contraction
    (fp32 operands hi/lo-split into fp16), ACT converts PSUM fp32 -> SBUF
    fp16 with Relu, DVE does the banded row-min (fused custom MINMIN op at
    2x) and a sliding col-min accumulator M (fp16 tensor_tensor at 2x).
    PE transposes finished M blocks mid-loop, DVE reduces -> col mins.
  - Host patch: points whose banded min exceeds the window's radial
    coverage (provable criterion) get exact numpy rows (~35/batch-side),
    making the result exact up to fp16 rounding.  cd/f1 host finalize.
"""

import sys

if "/opt/trn_rl_repo" not in sys.path:
    sys.path.insert(0, "/opt/trn_rl_repo")

from contextlib import ExitStack

import numpy as np

import concourse.tile as tile
import concourse.dve_ops as dve_ops
from concourse import bacc, bass_isa, mybir
from concourse.bass_utils import run_bass_kernel_spmd
from concourse.dve_spec import C0, AluOp, Spec, Src0, Src1, lower, minn
from concourse.dve_uop import DveOpSpec

F16 = mybir.dt.float16
F32 = mybir.dt.float32
MIN = mybir.AluOpType.min
AXX = mybir.AxisListType.X

SCALE = 24.0  # coordinate prescale; distances come out scaled by SCALE**2
DSCALE = SCALE * SCALE
F1_THRESHOLD = 1e-4
PAD_DIST = 60000.0  # scaled distance of padding columns (> max real ~53k)

N_CORES = 8
K_AUG = 13  # 9 coord-product rows + 2 sq1 rows + 2 sq2 rows
BAND_W = 448  # band width (columns per row-tile window)
import os as _os
USE_MINMIN_2X = _os.environ.get("MINMIN_2X", "0") == "1"

_RealCDA = bass_isa.InstCustomDveAnt


def _cda_perf(*a, **kw):
    """bass.py constructs InstCustomDveAnt via module attr; force perf_max=1
    (2x_1PORT opt-in, instruction byte 36[7:6]) for the MINMIN op."""
    inst = _RealCDA(*a, **kw)
    if USE_MINMIN_2X and kw.get("op_name") == "MINMIN_REDUCE_ANT":
        inst.perf_max = 1
    return inst


bass_isa.InstCustomDveAnt = _cda_perf


def _split16(v):
    """Split fp32 array into fp16 hi + lo so hi+lo ~= v to ~2^-22 rel."""
    hi = v.astype(np.float16)
    lo = (v - hi.astype(np.float32)).astype(np.float16)
    return hi, lo


def _aug_rows(xyz, side):
    """Augmented fp16 operand rows [K_AUG, n] for one cloud.

    side='lhs': rows for xyz1 (stationary), side='rhs': rows for xyz2
    (moving), such that sum_k lhsT[k,p]*rhs[k,f] ~= DSCALE*||x1_p - x2_f||^2.
    """
    n = xyz.shape[0]
    out = np.empty((K_AUG, n), np.float16)
    if side == "lhs":
        v = (-2.0 * SCALE) * xyz.astype(np.float32)
        h, l = _split16(v)
        sq = ((SCALE * xyz.astype(np.float32)) ** 2).sum(axis=1) * 0.25
        sh, sl = _split16(sq)
        for c in range(3):
            out[3 * c + 0] = h[:, c]
            out[3 * c + 1] = h[:, c]
            out[3 * c + 2] = l[:, c]
        out[9] = sh
        out[10] = sl
        out[11] = np.float16(4.0)
        out[12] = np.float16(4.0)
    else:
        w = SCALE * xyz.astype(np.float32)
        h, l = _split16(w)
        sq = ((w.astype(np.float64) ** 2).sum(axis=1) * 0.25).astype(np.float32)
        sh, sl = _split16(sq)
        for c in range(3):
            out[3 * c + 0] = h[:, c]
            out[3 * c + 1] = l[:, c]
            out[3 * c + 2] = h[:, c]
        out[9] = np.float16(4.0)
        out[10] = np.float16(4.0)
        out[11] = sh
        out[12] = sl
    return out


def _build_minmin_2x(uops_1x):
    """Hand-authored 2x_1PORT uop program for MINMIN (no stock accum op runs
    above 1x; this processes 2 packed fp16 pairs per port per cycle).

    steady-state, per cycle:
      lanes: 1=SRC_0 2=SRC_1 3=SRC_0_HI 4=SRC_1_HI
      s0: lo = MIN(SRC_0, SRC_1); delay lanes carry the HI pair forward
      s1: hi = MIN(SRC_0_HI, SRC_1_HI); delay0 captures s0's lo (realign)
      s2: merged = MIN(hi[curr], lo[delay0])
      s3: acc = MIN(merged[curr], acc[self])  (alu_out_a = accumulator)
      s4-7: hold.  out tensor gets scratch values; only accum_out is used.
    """
    from copy import deepcopy

    from concourse.dve_uop import (
        AluInp, DelayInp, InpSel, OutPath, OutSel, Trigger,
        UopConfig, UopDpConfig,
    )
    from concourse.dve_uop import AluOp as UAluOp

    PD = DelayInp.PREV_DELAY
    PAO = DelayInp.PREV_ALU_OUT

    def dp(op, s0, s1, a=0):
        return UopDpConfig(
            op=op, alu_src0=s0, alu_src1=s1,
            delay=[PD, PD, PD, PD, PAO, PAO, PAO],
            alu_out_enable=1, swap_enable=0,
            alu_out_a_enable=a, alu_out_b_enable=0,
            delay_enable=[1, 0, 0, 0, 0, 0, 0],
            idx0_sel=0, idx1_sel=0,
        )

    s0 = dp(UAluOp.MIN, AluInp.PREV_DELAY_0, AluInp.PREV_DELAY_1)
    s0.delay_enable = [1, 1, 1, 1, 0, 0, 0]
    s1 = dp(UAluOp.MIN, AluInp.PREV_DELAY_2, AluInp.PREV_DELAY_3)
    s1.delay = [PAO, PD, PD, PD, PAO, PAO, PAO]
    stages = [
        s0,
        s1,
        # s2: PREV_ALU_OUT = s1's hi (prev block), PREV_DELAY_0 = lo
        dp(UAluOp.MIN, AluInp.PREV_ALU_OUT, AluInp.PREV_DELAY_0),
        # s3: CURR_ALU_OUT = own flop = the accumulator (seeded with C0)
        dp(UAluOp.MIN, AluInp.CURR_ALU_OUT, AluInp.PREV_ALU_OUT, a=1),
        dp(UAluOp.BYPASS, AluInp.PREV_ALU_OUT, AluInp.PREV_ALU_OUT, a=1),
        dp(UAluOp.BYPASS, AluInp.PREV_ALU_OUT, AluInp.PREV_ALU_OUT, a=1),
        dp(UAluOp.BYPASS, AluInp.PREV_ALU_OUT, AluInp.PREV_ALU_OUT, a=1),
        dp(UAluOp.BYPASS, AluInp.PREV_ALU_OUT, AluInp.PREV_ALU_OUT, a=1),
    ]

    def seed_dp():
        # seed token (1 cycle): C0 enters on input lane 3, rides delay
        # lane 2 to stage 3 which loads the accumulator flop; stage 0
        # bypasses C0 so stage 1's delay lane 0 (the merge operand in the
        # steady state) also starts at C0 instead of stale garbage.
        st0 = dp(UAluOp.BYPASS, AluInp.PREV_DELAY_2, AluInp.PREV_DELAY_2)
        st0.delay_enable = [1, 1, 1, 0, 0, 0, 0]
        st1 = dp(UAluOp.BYPASS, AluInp.PREV_ALU_OUT, AluInp.PREV_ALU_OUT)
        st1.delay = [PAO, PD, PD, PD, PAO, PAO, PAO]
        st1.delay_enable = [1, 0, 1, 0, 0, 0, 0]
        st2 = dp(UAluOp.BYPASS, AluInp.PREV_ALU_OUT, AluInp.PREV_ALU_OUT)
        st2.delay_enable = [1, 0, 1, 0, 0, 0, 0]
        st3 = dp(UAluOp.BYPASS, AluInp.PREV_DELAY_2, AluInp.PREV_DELAY_2, a=1)
        return [st0, st1, st2, st3] + [
            dp(UAluOp.BYPASS, AluInp.PREV_ALU_OUT, AluInp.PREV_ALU_OUT, a=1)
            for _ in range(4)
        ]
    steady = UopConfig(
        inp=[InpSel.ZERO, InpSel.SRC_0, InpSel.SRC_1, InpSel.SRC_0_HI,
             InpSel.SRC_1_HI, InpSel.ZERO, InpSel.ZERO, InpSel.ZERO],
        inp_enable=[0, 1, 1, 1, 1, 0, 0, 0],
        out={OutPath.WR0_LO: OutSel.ALU_OUT, OutPath.WR0_HI: OutSel.DELAY_0,
             OutPath.WR1_LO: OutSel.ALU_OUT, OutPath.WR1_HI: OutSel.ALU_OUT},
        out_enable={OutPath.WR0_LO: 1, OutPath.WR0_HI: 1,
                    OutPath.WR1_LO: 0, OutPath.WR1_HI: 0},
        out_last_subdim_enable=0,
        force_two_data_zero=0, force_two_data_one=0,
        require_inp0=1, require_inp1=1,
        repeat_count=0,
        trigger=(Trigger.SRC_TENSOR_DONE, Trigger.NONE, Trigger.NONE),
        next_uop=(0, 0, 0),
        inc_parameter_index=0, enable_rev_ops=0,
        match_mask=0, valid_match=0, replace_on_match=0, clear_match=0,
        write_predicate_select=0, write_predicate_enable=0,
        delay_shift8=0, index_increment=0, index_clear=0,
        accum_enabled=1, v4={},
        datapath_config=stages,
    )
    seed = deepcopy(uops_1x[0])
    seed.datapath_config = seed_dp()
    return [seed, steady]


def _register_minmin_reduce():
    """Custom DVE op: out = min(in0, in1); accum_out = min-fold(out)."""
    name = "MINMIN_REDUCE_ANT"
    if name in dve_ops._SUB_OPCODE_FOR_NAME:
        return next(op for op in dve_ops.OPS if op.name == name)

    def _ref(in0, in1, c0, c1, c2):
        out = np.minimum(np.asarray(in0, np.float32), np.asarray(in1, np.float32))
        acc = out.reshape(out.shape[0], -1).min(axis=-1, keepdims=True)
        acc = np.minimum(acc, c0)
        return out, acc

    spec = Spec(body=minn(Src0, Src1), accum=AluOp.MIN, accum_init=C0,
                reference=_ref)
    row = max(dve_ops._SUB_OPCODE_FOR_NAME.values()) + 1
    u1 = lower(spec, ver="v3")
    s3 = DveOpSpec(name=name, opcode=row, uops=u1,
                   uops_2x=_build_minmin_2x(u1) if USE_MINMIN_2X else None,
                   rd1_en=True, perf_max=1 if USE_MINMIN_2X else 0)
    s3.validate("v3")
    shas = {"v3": s3.sha("v3")}
    try:
        u1v4 = lower(spec, ver="v4")
        s4 = DveOpSpec(name=name, opcode=row, uops=u1v4, rd1_en=True)
        shas["v4"] = s4.sha("v4")
        dve_ops._COMPILE_CACHE[(name, "v4")] = s4
    except Exception:
        pass
    op = dve_ops.DveOp(name, spec, subdim=False, uops_sha=shas)
    dve_ops._COMPILE_CACHE[(name, "v3")] = s3
    dve_ops.OPS.append(op)
    dve_ops.CUSTOM_DVE_SPECS[name] = spec
    dve_ops._SUB_OPCODE_FOR_NAME[name] = row
    return op


def build_program(n_rows=4096, W=BAND_W):
    """Per-core banded program (SPMD-identical across cores).

    Local column space C = n_rows + W; tile t (128 rows) sees window
    [128*t, 128*t + W).  The host pre-shifts/pads each core's rhs so this
    static window pattern is centred on the tile's radius range.  The
    column-min output is the raw fp16 accumulator M, DMA'd out in chunks
    as its blocks finalize; the host does the 128-way partition min.
    """
    ROWT = n_rows // 128
    C = n_rows + W
    PER = 8  # tiles per M-chunk DMA-out
    NMM = (W + 511) // 512
    PSW = 512 * NMM  # psum slot width per tile (bank aligned)

    MINMIN = _register_minmin_reduce()
    nc = bacc.Bacc("TRN2", target_bir_lowering=False, debug=False,
                   num_devices=N_CORES)
    lhsT_d = nc.dram_tensor("lhsT", [K_AUG, n_rows], F16, kind="ExternalInput").ap()
    rhs_d = nc.dram_tensor("rhs", [K_AUG, C], F16, kind="ExternalInput").ap()
    out1_d = nc.dram_tensor("out1", [128, ROWT], F32, kind="ExternalOutput").ap()
    out2_d = nc.dram_tensor("out2", [128, C], F16, kind="ExternalOutput").ap()

    with tile.TileContext(nc) as tc, ExitStack() as ctx:
        const = ctx.enter_context(tc.tile_pool(name="const", bufs=1))
        d16p = ctx.enter_context(tc.tile_pool(name="d16", bufs=3))
        mp = ctx.enter_context(tc.tile_pool(name="macc", bufs=1))
        treep = ctx.enter_context(tc.tile_pool(name="tree", bufs=2))
        outp = ctx.enter_context(tc.tile_pool(name="outs", bufs=1))
        psp = ctx.enter_context(tc.tile_pool(name="ps", bufs=2, space="PSUM"))

        M = mp.tile([128, C], F16)
        R = outp.tile([128, ROWT], F32)

        # init col-min accumulator while input DMAs are in flight
        nc.vector.memset(M[:, 0:C // 2], PAD_DIST)
        nc.vector.memset(M[:, C // 2:C], PAD_DIST)

        # inputs: split across the two HWDGE queues (sync + scalar), in
        # first-needed-first order so the pipeline lights up early.
        w_sb = const.tile([K_AUG, n_rows], F16)
        r_sb = const.tile([K_AUG, C], F16)
        nc.sync.dma_start(r_sb[:, 0:W + 256], rhs_d[:, 0:W + 256])
        nc.scalar.dma_start(w_sb[:, 0:512], lhsT_d[:, 0:512])
        s = W + 256
        while s < C:
            w = min(1536, C - s)
            nc.sync.dma_start(r_sb[:, s:s + w], rhs_d[:, s:s + w])
            s += w
        s = 512
        while s < n_rows:
            w = min(1024, n_rows - s)
            nc.scalar.dma_start(w_sb[:, s:s + w], lhsT_d[:, s:s + w])
            s += w

        half = W // 2
        GRP = 4  # tiles per PSUM allocation / ACTIVATE
        for tp in range(ROWT // GRP):
            ps = psp.tile([128, GRP * PSW], F32, tag="ps")
            for i in range(GRP):
                t = GRP * tp + i
                for j in range(NMM):
                    wj = min(512, W - 512 * j)
                    nc.tensor.matmul(
                        ps[:, PSW * i + 512 * j:PSW * i + 512 * j + wj],
                        w_sb[:, 128 * t:128 * (t + 1)],
                        r_sb[:, 128 * t + 512 * j:128 * t + 512 * j + wj],
                        start=True, stop=True,
                    )
            d16 = d16p.tile([128, GRP * W], F16, tag="d16")
            nc.scalar.activation(
                d16[:].rearrange("p (i w) -> p i w", i=GRP),
                ps[:].rearrange("p (i w) -> p i w", i=GRP)[:, :, 0:W],
                mybir.ActivationFunctionType.Relu,
            )
            for i in range(GRP):
                t = GRP * tp + i
                lo = 128 * t
                dt = d16[:, W * i:W * (i + 1)]
                nc.vector.tensor_tensor(M[:, lo:lo + W], M[:, lo:lo + W],
                                        dt, op=MIN)
                u = treep.tile([128, half], F16, tag="mm_scratch")
                nc.vector._custom_dve(
                    MINMIN, out=u[:], in0=dt[:, 0:half], in1=dt[:, half:W],
                    s0=PAD_DIST, accum_out=R[:, t:t + 1],
                )
            # M cols [0, 128*t+128) are final after tile t: stream finished
            # chunks to DRAM during the loop (host does the partition-min).
            t = GRP * tp + GRP - 1
            if (t + 1) % PER == 0:
                q = (t + 1) // PER - 1
                c0, c1 = q * PER * 128, (q + 1) * PER * 128
                nc.sync.dma_start(out2_d[:, c0:c1], M[:, c0:c1])

        c0 = (ROWT // PER) * PER * 128
        nc.sync.dma_start(out2_d[:, c0:C], M[:, c0:C])
        nc.sync.dma_start(out1_d, R[:])

    nc.compile()
    return nc


_CACHE = {}


def _get_program(n_rows, W):
    key = (n_rows, W)
    if key not in _CACHE:
        _CACHE[key] = build_program(n_rows, W)
    return _CACHE[key]


def run_device(xyz1, xyz2, trace=False):
    """Run the 8-core SPMD banded program; returns (dist1, dist2, res),
    dist1/dist2 in original (unsorted) order, exact after host patch."""
    xyz1 = np.asarray(xyz1)
    xyz2 = np.asarray(xyz2)
    B, N, _ = xyz1.shape
    M = xyz2.shape[1]
    W = BAND_W
    halves = N_CORES // B  # row-halves per batch (2)
    n_rows = N // halves
    C = n_rows + W
    nc = _get_program(n_rows, W)

    # host prep: radius sort, augmented operands, per-core shifted rhs
    perms1, perms2 = [], []
    a_s, c_s, ra_s, rc_s = [], [], [], []
    rhs_aug = []
    for b in range(B):
        r1 = np.linalg.norm(xyz1[b].astype(np.float64), axis=1)
        r2 = np.linalg.norm(xyz2[b].astype(np.float64), axis=1)
        p1 = np.argsort(r1, kind="stable")
        p2 = np.argsort(r2, kind="stable")
        perms1.append(p1); perms2.append(p2)
        a_s.append(xyz1[b][p1]); c_s.append(xyz2[b][p2])
        ra_s.append(r1[p1]); rc_s.append(r2[p2])
        rhs_aug.append(_aug_rows(c_s[b], "rhs"))

    # padding column (far away): contributes 4*sh = PAD_DIST
    pad_col = np.zeros((K_AUG,), np.float16)
    pad_col[9] = np.float16(4.0)
    pad_col[10] = np.float16(4.0)
    pad_col[11] = np.float16(PAD_DIST / 4.0)
    pad_col[12] = np.float16(0.0)

    in_maps = []
    bases = []
    for c in range(N_CORES):
        b, h = divmod(c, halves)
        lhsT = _aug_rows(a_s[b][h * n_rows:(h + 1) * n_rows], "lhs")
        base = h * n_rows + 64 - W // 2  # global col of local col 0
        bases.append(base)
        rhs = np.repeat(pad_col[:, None], C, axis=1).astype(np.float16)
        g0, g1 = max(base, 0), min(base + C, M)
        rhs[:, g0 - base:g1 - base] = rhs_aug[b][:, g0:g1]
        in_maps.append({"lhsT": lhsT, "rhs": rhs})

    res = run_bass_kernel_spmd(nc, in_maps, list(range(N_CORES)), trace=trace)

    ROWT = n_rows // 128
    COV = 128 * (ROWT - 1) + W
    dist1_s = np.empty((B, N), np.float64)
    dist2_s = np.full((B, M), np.inf, np.float64)
    for c in range(N_CORES):
        b, h = divmod(c, halves)
        o1 = res.results[c]["out1"].astype(np.float64)  # [128, ROWT]
        o2 = res.results[c]["out2"]  # [128, C] fp16 raw col-min accumulator
        dist1_s[b, h * n_rows:(h + 1) * n_rows] = o1.T.reshape(-1)
        colmin = o2.astype(np.float32).min(axis=0).astype(np.float64)
        j_loc = np.arange(C)
        cols = bases[c] + j_loc
        valid = (cols >= 0) & (cols < M) & (j_loc < COV)
        np.minimum.at(dist2_s[b], cols[valid], colmin[valid])
    dist1_s /= DSCALE
    dist2_s /= DSCALE

    # --- exact host patch for at-risk points -----------------------------
    # Window of global tile T (row range [128T,128T+128)): global cols
    # [64 - W/2 + 128T, 64 + W/2 + 128T) intersected with [0, M).
    nt = N // 128
    w_lo = np.maximum(64 - W // 2 + 128 * np.arange(nt), 0)
    w_hi = np.minimum(64 + W // 2 + 128 * np.arange(nt), M)
    for b in range(B):
        ra, rc = ra_s[b], rc_s[b]
        a, cc = a_s[b], c_s[b]
        # coverage radius per sorted row: window covers rc[w_lo[t]..w_hi[t]-1]
        cov1 = np.empty(N)
        for t in range(nt):
            lo = -np.inf if w_lo[t] == 0 else rc[w_lo[t]]
            hi = np.inf if w_hi[t] == M else rc[w_hi[t] - 1]
            rr = ra[128 * t:128 * (t + 1)]
            cov1[128 * t:128 * (t + 1)] = np.minimum(rr - lo, hi - rr)
        # coverage per sorted col: tiles t with w_lo[t] <= j < w_hi[t] form a
        # contiguous range; their rows span a contiguous sorted-row range.
        j_all = np.arange(M)
        ft = np.searchsorted(w_hi - 1, j_all, "left")   # first tile covering j
        lt = np.searchsorted(w_lo, j_all, "right") - 1  # last tile covering j
        lo_r = np.where(ft <= 0, -np.inf, ra[np.minimum(ft * 128, N - 1)])
        hi_r = np.where(lt >= nt - 1, np.inf, ra[np.minimum((lt + 1) * 128 - 1, N - 1)])
        cov2 = np.minimum(rc - lo_r, hi_r - rc)
        risk1 = np.where(dist1_s[b] > cov1 ** 2 * 0.997 - 1e-6)[0]
        risk2 = np.where(dist2_s[b] > cov2 ** 2 * 0.997 - 1e-6)[0]
        if len(risk1):
            d = ((a[risk1][:, None, :].astype(np.float64)
                  - cc[None, :, :].astype(np.float64)) ** 2).sum(-1)
            dist1_s[b][risk1] = d.min(1)
        if len(risk2):
            d = ((cc[risk2][:, None, :].astype(np.float64)
                  - a[None, :, :].astype(np.float64)) ** 2).sum(-1)
            dist2_s[b][risk2] = d.min(1)

    # unsort back to original order
    dist1 = np.empty_like(dist1_s)
    dist2 = np.empty_like(dist2_s)
    for b in range(B):
        dist1[b][perms1[b]] = dist1_s[b]
        dist2[b][perms2[b]] = dist2_s[b]
    return dist1, dist2, res


def _finalize(dist1, dist2):
    dist1 = np.maximum(dist1, 0.0)
    dist2 = np.maximum(dist2, 0.0)
    cd_p = (np.sqrt(dist1).mean(axis=1) + np.sqrt(dist2).mean(axis=1)) / 2.0
    cd_t = dist1.mean(axis=1) + dist2.mean(axis=1)
    p1 = (dist1 < F1_THRESHOLD).mean(axis=1)
    p2 = (dist2 < F1_THRESHOLD).mean(axis=1)
    denom = p1 + p2
    f1 = np.where(denom > 0, 2.0 * p1 * p2 / np.where(denom > 0, denom, 1.0), 0.0)
    return (cd_p.astype(np.float32), cd_t.astype(np.float32),
            f1.astype(np.float32))


def kernel(xyz1, xyz2):
    dist1, dist2, _ = run_device(xyz1, xyz2, trace=False)
    return _finalize(dist1, dist2)


# revision 19
# speedup vs baseline: 1.8040x; 1.0765x over previous
"""Chamfer distance + F1 kernel for Trainium2 (8 NeuronCores).

Banded-KNN strategy (B=4 batches, N=M=8192 points, 3D):
  - Host sorts each batch's clouds by radius r=||p||.  Since radius is
    1-Lipschitz (|r_x - r_y| <= ||x-y||), a point's nearest neighbour is
    radially close whenever it is close in space, so a narrow band around
    the radius-sorted diagonal contains the NN for all but a handful of
    isolated points.
  - core c handles batch b = c//2, sorted-row-half h = c%2 (4096 rows).
    Each 128-row tile t computes scaled squared distances only against a
    static W-wide window of radius-sorted xyz2 (window slides 128 cols per
    tile).  The host hands each core a pre-shifted, padded slice of sorted
    xyz2 so the compiled program is identical across cores (SPMD).
  - PE computes the W-wide block in fp16 via the augmented K=13 contraction
    (fp32 operands hi/lo-split into fp16), ACT converts PSUM fp32 -> SBUF
    fp16 with Relu, DVE does the banded row-min (fused custom MINMIN op at
    2x) and a sliding col-min accumulator M (fp16 tensor_tensor at 2x).
    PE transposes finished M blocks mid-loop, DVE reduces -> col mins.
  - Host patch: points whose banded min exceeds the window's radial
    coverage (provable criterion) get exact numpy rows (~35/batch-side),
    making the result exact up to fp16 rounding.  cd/f1 host finalize.
"""

import sys

if "/opt/trn_rl_repo" not in sys.path:
    sys.path.insert(0, "/opt/trn_rl_repo")

from contextlib import ExitStack

import numpy as np

import concourse.tile as tile
import concourse.dve_ops as dve_ops
from concourse import bacc, bass_isa, mybir
from concourse.bass_utils import run_bass_kernel_spmd
from concourse.dve_spec import C0, AluOp, Spec, Src0, Src1, lower, minn
from concourse.dve_uop import DveOpSpec

F16 = mybir.dt.float16
F32 = mybir.dt.float32
MIN = mybir.AluOpType.min
AXX = mybir.AxisListType.X

SCALE = 24.0  # coordinate prescale; distances come out scaled by SCALE**2
DSCALE = SCALE * SCALE
F1_THRESHOLD = 1e-4
PAD_DIST = 60000.0  # scaled distance of padding columns (> max real ~53k)

N_CORES = 8
K_AUG = 13  # 9 coord-product rows + 2 sq1 rows + 2 sq2 rows
BAND_W = 384  # band width (columns per row-tile window)
import os as _os
USE_MINMIN_2X = _os.environ.get("MINMIN_2X", "0") == "1"

_RealCDA = bass_isa.InstCustomDveAnt


def _cda_perf(*a, **kw):
    """bass.py constructs InstCustomDveAnt via module attr; force perf_max=1
    (2x_1PORT opt-in, instruction byte 36[7:6]) for the MINMIN op."""
    inst = _RealCDA(*a, **kw)
    if USE_MINMIN_2X and kw.get("op_name") == "MINMIN_REDUCE_ANT":
        inst.perf_max = 1
    return inst


bass_isa.InstCustomDveAnt = _cda_perf


def _split16(v):
    """Split fp32 array into fp16 hi + lo so hi+lo ~= v to ~2^-22 rel."""
    hi = v.astype(np.float16)
    lo = (v - hi.astype(np.float32)).astype(np.float16)
    return hi, lo


def _aug_rows(xyz, side):
    """Augmented fp16 operand rows [K_AUG, n] for one cloud.

    side='lhs': rows for xyz1 (stationary), side='rhs': rows for xyz2
    (moving), such that sum_k lhsT[k,p]*rhs[k,f] ~= DSCALE*||x1_p - x2_f||^2.
    """
    n = xyz.shape[0]
    out = np.empty((K_AUG, n), np.float16)
    if side == "lhs":
        v = (-2.0 * SCALE) * xyz.astype(np.float32)
        h, l = _split16(v)
        sq = ((SCALE * xyz.astype(np.float32)) ** 2).sum(axis=1) * 0.25
        sh, sl = _split16(sq)
        for c in range(3):
            out[3 * c + 0] = h[:, c]
            out[3 * c + 1] = h[:, c]
            out[3 * c + 2] = l[:, c]
        out[9] = sh
        out[10] = sl
        out[11] = np.float16(4.0)
        out[12] = np.float16(4.0)
    else:
        w = SCALE * xyz.astype(np.float32)
        h, l = _split16(w)
        sq = ((w.astype(np.float64) ** 2).sum(axis=1) * 0.25).astype(np.float32)
        sh, sl = _split16(sq)
        for c in range(3):
            out[3 * c + 0] = h[:, c]
            out[3 * c + 1] = l[:, c]
            out[3 * c + 2] = h[:, c]
        out[9] = np.float16(4.0)
        out[10] = np.float16(4.0)
        out[11] = sh
        out[12] = sl
    return out


def _build_minmin_2x(uops_1x):
    """Hand-authored 2x_1PORT uop program for MINMIN (no stock accum op runs
    above 1x; this processes 2 packed fp16 pairs per port per cycle).

    steady-state, per cycle:
      lanes: 1=SRC_0 2=SRC_1 3=SRC_0_HI 4=SRC_1_HI
      s0: lo = MIN(SRC_0, SRC_1); delay lanes carry the HI pair forward
      s1: hi = MIN(SRC_0_HI, SRC_1_HI); delay0 captures s0's lo (realign)
      s2: merged = MIN(hi[curr], lo[delay0])
      s3: acc = MIN(merged[curr], acc[self])  (alu_out_a = accumulator)
      s4-7: hold.  out tensor gets scratch values; only accum_out is used.
    """
    from copy import deepcopy

    from concourse.dve_uop import (
        AluInp, DelayInp, InpSel, OutPath, OutSel, Trigger,
        UopConfig, UopDpConfig,
    )
    from concourse.dve_uop import AluOp as UAluOp

    PD = DelayInp.PREV_DELAY
    PAO = DelayInp.PREV_ALU_OUT

    def dp(op, s0, s1, a=0):
        return UopDpConfig(
            op=op, alu_src0=s0, alu_src1=s1,
            delay=[PD, PD, PD, PD, PAO, PAO, PAO],
            alu_out_enable=1, swap_enable=0,
            alu_out_a_enable=a, alu_out_b_enable=0,
            delay_enable=[1, 0, 0, 0, 0, 0, 0],
            idx0_sel=0, idx1_sel=0,
        )

    s0 = dp(UAluOp.MIN, AluInp.PREV_DELAY_0, AluInp.PREV_DELAY_1)
    s0.delay_enable = [1, 1, 1, 1, 0, 0, 0]
    s1 = dp(UAluOp.MIN, AluInp.PREV_DELAY_2, AluInp.PREV_DELAY_3)
    s1.delay = [PAO, PD, PD, PD, PAO, PAO, PAO]
    stages = [
        s0,
        s1,
        # s2: PREV_ALU_OUT = s1's hi (prev block), PREV_DELAY_0 = lo
        dp(UAluOp.MIN, AluInp.PREV_ALU_OUT, AluInp.PREV_DELAY_0),
        # s3: CURR_ALU_OUT = own flop = the accumulator (seeded with C0)
        dp(UAluOp.MIN, AluInp.CURR_ALU_OUT, AluInp.PREV_ALU_OUT, a=1),
        dp(UAluOp.BYPASS, AluInp.PREV_ALU_OUT, AluInp.PREV_ALU_OUT, a=1),
        dp(UAluOp.BYPASS, AluInp.PREV_ALU_OUT, AluInp.PREV_ALU_OUT, a=1),
        dp(UAluOp.BYPASS, AluInp.PREV_ALU_OUT, AluInp.PREV_ALU_OUT, a=1),
        dp(UAluOp.BYPASS, AluInp.PREV_ALU_OUT, AluInp.PREV_ALU_OUT, a=1),
    ]

    def seed_dp():
        # seed token (1 cycle): C0 enters on input lane 3, rides delay
        # lane 2 to stage 3 which loads the accumulator flop; stage 0
        # bypasses C0 so stage 1's delay lane 0 (the merge operand in the
        # steady state) also starts at C0 instead of stale garbage.
        st0 = dp(UAluOp.BYPASS, AluInp.PREV_DELAY_2, AluInp.PREV_DELAY_2)
        st0.delay_enable = [1, 1, 1, 0, 0, 0, 0]
        st1 = dp(UAluOp.BYPASS, AluInp.PREV_ALU_OUT, AluInp.PREV_ALU_OUT)
        st1.delay = [PAO, PD, PD, PD, PAO, PAO, PAO]
        st1.delay_enable = [1, 0, 1, 0, 0, 0, 0]
        st2 = dp(UAluOp.BYPASS, AluInp.PREV_ALU_OUT, AluInp.PREV_ALU_OUT)
        st2.delay_enable = [1, 0, 1, 0, 0, 0, 0]
        st3 = dp(UAluOp.BYPASS, AluInp.PREV_DELAY_2, AluInp.PREV_DELAY_2, a=1)
        return [st0, st1, st2, st3] + [
            dp(UAluOp.BYPASS, AluInp.PREV_ALU_OUT, AluInp.PREV_ALU_OUT, a=1)
            for _ in range(4)
        ]
    steady = UopConfig(
        inp=[InpSel.ZERO, InpSel.SRC_0, InpSel.SRC_1, InpSel.SRC_0_HI,
             InpSel.SRC_1_HI, InpSel.ZERO, InpSel.ZERO, InpSel.ZERO],
        inp_enable=[0, 1, 1, 1, 1, 0, 0, 0],
        out={OutPath.WR0_LO: OutSel.ALU_OUT, OutPath.WR0_HI: OutSel.DELAY_0,
             OutPath.WR1_LO: OutSel.ALU_OUT, OutPath.WR1_HI: OutSel.ALU_OUT},
        out_enable={OutPath.WR0_LO: 1, OutPath.WR0_HI: 1,
                    OutPath.WR1_LO: 0, OutPath.WR1_HI: 0},
        out_last_subdim_enable=0,
        force_two_data_zero=0, force_two_data_one=0,
        require_inp0=1, require_inp1=1,
        repeat_count=0,
        trigger=(Trigger.SRC_TENSOR_DONE, Trigger.NONE, Trigger.NONE),
        next_uop=(0, 0, 0),
        inc_parameter_index=0, enable_rev_ops=0,
        match_mask=0, valid_match=0, replace_on_match=0, clear_match=0,
        write_predicate_select=0, write_predicate_enable=0,
        delay_shift8=0, index_increment=0, index_clear=0,
        accum_enabled=1, v4={},
        datapath_config=stages,
    )
    seed = deepcopy(uops_1x[0])
    seed.datapath_config = seed_dp()
    return [seed, steady]


def _register_minmin_reduce():
    """Custom DVE op: out = min(in0, in1); accum_out = min-fold(out)."""
    name = "MINMIN_REDUCE_ANT"
    if name in dve_ops._SUB_OPCODE_FOR_NAME:
        return next(op for op in dve_ops.OPS if op.name == name)

    def _ref(in0, in1, c0, c1, c2):
        out = np.minimum(np.asarray(in0, np.float32), np.asarray(in1, np.float32))
        acc = out.reshape(out.shape[0], -1).min(axis=-1, keepdims=True)
        acc = np.minimum(acc, c0)
        return out, acc

    spec = Spec(body=minn(Src0, Src1), accum=AluOp.MIN, accum_init=C0,
                reference=_ref)
    row = max(dve_ops._SUB_OPCODE_FOR_NAME.values()) + 1
    u1 = lower(spec, ver="v3")
    s3 = DveOpSpec(name=name, opcode=row, uops=u1,
                   uops_2x=_build_minmin_2x(u1) if USE_MINMIN_2X else None,
                   rd1_en=True, perf_max=1 if USE_MINMIN_2X else 0)
    s3.validate("v3")
    shas = {"v3": s3.sha("v3")}
    try:
        u1v4 = lower(spec, ver="v4")
        s4 = DveOpSpec(name=name, opcode=row, uops=u1v4, rd1_en=True)
        shas["v4"] = s4.sha("v4")
        dve_ops._COMPILE_CACHE[(name, "v4")] = s4
    except Exception:
        pass
    op = dve_ops.DveOp(name, spec, subdim=False, uops_sha=shas)
    dve_ops._COMPILE_CACHE[(name, "v3")] = s3
    dve_ops.OPS.append(op)
    dve_ops.CUSTOM_DVE_SPECS[name] = spec
    dve_ops._SUB_OPCODE_FOR_NAME[name] = row
    return op


def build_program(n_rows=4096, W=BAND_W):
    """Per-core banded program (SPMD-identical across cores).

    Local column space C = n_rows + W; tile t (128 rows) sees window
    [128*t, 128*t + W).  The host pre-shifts/pads each core's rhs so this
    static window pattern is centred on the tile's radius range.  The
    column-min output is the raw fp16 accumulator M, DMA'd out in chunks
    as its blocks finalize; the host does the 128-way partition min.
    """
    ROWT = n_rows // 128
    C = n_rows + W
    PER = 8  # tiles per M-chunk DMA-out
    NMM = (W + 511) // 512
    PSW = 512 * NMM  # psum slot width per tile (bank aligned)

    MINMIN = _register_minmin_reduce()
    nc = bacc.Bacc("TRN2", target_bir_lowering=False, debug=False,
                   num_devices=N_CORES)
    lhsT_d = nc.dram_tensor("lhsT", [K_AUG, n_rows], F16, kind="ExternalInput").ap()
    rhs_d = nc.dram_tensor("rhs", [K_AUG, C], F16, kind="ExternalInput").ap()
    out1_d = nc.dram_tensor("out1", [128, ROWT], F32, kind="ExternalOutput").ap()
    out2_d = nc.dram_tensor("out2", [128, C], F16, kind="ExternalOutput").ap()

    with tile.TileContext(nc) as tc, ExitStack() as ctx:
        const = ctx.enter_context(tc.tile_pool(name="const", bufs=1))
        d16p = ctx.enter_context(tc.tile_pool(name="d16", bufs=3))
        mp = ctx.enter_context(tc.tile_pool(name="macc", bufs=1))
        treep = ctx.enter_context(tc.tile_pool(name="tree", bufs=2))
        outp = ctx.enter_context(tc.tile_pool(name="outs", bufs=1))
        psp = ctx.enter_context(tc.tile_pool(name="ps", bufs=2, space="PSUM"))

        M = mp.tile([128, C], F16)
        R = outp.tile([128, ROWT], F32)

        # init col-min accumulator while input DMAs are in flight
        nc.vector.memset(M[:, 0:C // 2], PAD_DIST)
        nc.vector.memset(M[:, C // 2:C], PAD_DIST)

        # inputs: split across the two HWDGE queues (sync + scalar), in
        # first-needed-first order so the pipeline lights up early.
        w_sb = const.tile([K_AUG, n_rows], F16)
        r_sb = const.tile([K_AUG, C], F16)
        nc.sync.dma_start(r_sb[:, 0:W + 512], rhs_d[:, 0:W + 512])
        nc.scalar.dma_start(w_sb[:, 0:512], lhsT_d[:, 0:512])
        s = W + 512
        while s < C:
            w = min(1536, C - s)
            nc.sync.dma_start(r_sb[:, s:s + w], rhs_d[:, s:s + w])
            s += w
        s = 512
        while s < n_rows:
            w = min(1024, n_rows - s)
            nc.scalar.dma_start(w_sb[:, s:s + w], lhsT_d[:, s:s + w])
            s += w

        half = W // 2
        GRP = 4  # tiles per PSUM allocation / ACTIVATE
        for tp in range(ROWT // GRP):
            ps = psp.tile([128, GRP * PSW], F32, tag="ps")
            for i in range(GRP):
                t = GRP * tp + i
                for j in range(NMM):
                    wj = min(512, W - 512 * j)
                    nc.tensor.matmul(
                        ps[:, PSW * i + 512 * j:PSW * i + 512 * j + wj],
                        w_sb[:, 128 * t:128 * (t + 1)],
                        r_sb[:, 128 * t + 512 * j:128 * t + 512 * j + wj],
                        start=True, stop=True,
                    )
            d16 = d16p.tile([128, GRP * W], F16, tag="d16")
            nc.scalar.activation(
                d16[:].rearrange("p (i w) -> p i w", i=GRP),
                ps[:].rearrange("p (i w) -> p i w", i=GRP)[:, :, 0:W],
                mybir.ActivationFunctionType.Relu,
            )
            for i in range(GRP):
                t = GRP * tp + i
                lo = 128 * t
                dt = d16[:, W * i:W * (i + 1)]
                nc.vector.tensor_tensor(M[:, lo:lo + W], M[:, lo:lo + W],
                                        dt, op=MIN)
                u = treep.tile([128, half], F16, tag="mm_scratch")
                nc.vector._custom_dve(
                    MINMIN, out=u[:], in0=dt[:, 0:half], in1=dt[:, half:W],
                    s0=PAD_DIST, accum_out=R[:, t:t + 1],
                )
            # M cols [0, 128*t+128) are final after tile t: stream finished
            # chunks to DRAM during the loop (host does the partition-min).
            t = GRP * tp + GRP - 1
            if (t + 1) % PER == 0:
                q = (t + 1) // PER - 1
                c0, c1 = q * PER * 128, (q + 1) * PER * 128
                nc.sync.dma_start(out2_d[:, c0:c1], M[:, c0:c1])

        c0 = (ROWT // PER) * PER * 128
        nc.sync.dma_start(out2_d[:, c0:C], M[:, c0:C])
        nc.sync.dma_start(out1_d, R[:])

    nc.compile()
    return nc


_CACHE = {}


def _get_program(n_rows, W):
    key = (n_rows, W)
    if key not in _CACHE:
        _CACHE[key] = build_program(n_rows, W)
    return _CACHE[key]


def run_device(xyz1, xyz2, trace=False):
    """Run the 8-core SPMD banded program; returns (dist1, dist2, res),
    dist1/dist2 in original (unsorted) order, exact after host patch."""
    xyz1 = np.asarray(xyz1)
    xyz2 = np.asarray(xyz2)
    B, N, _ = xyz1.shape
    M = xyz2.shape[1]
    W = BAND_W
    halves = N_CORES // B  # row-halves per batch (2)
    n_rows = N // halves
    C = n_rows + W
    nc = _get_program(n_rows, W)

    # host prep: radius sort, augmented operands, per-core shifted rhs
    perms1, perms2 = [], []
    a_s, c_s, ra_s, rc_s = [], [], [], []
    rhs_aug = []
    for b in range(B):
        r1 = np.linalg.norm(xyz1[b].astype(np.float64), axis=1)
        r2 = np.linalg.norm(xyz2[b].astype(np.float64), axis=1)
        p1 = np.argsort(r1, kind="stable")
        p2 = np.argsort(r2, kind="stable")
        perms1.append(p1); perms2.append(p2)
        a_s.append(xyz1[b][p1]); c_s.append(xyz2[b][p2])
        ra_s.append(r1[p1]); rc_s.append(r2[p2])
        rhs_aug.append(_aug_rows(c_s[b], "rhs"))

    # padding column (far away): contributes 4*sh = PAD_DIST
    pad_col = np.zeros((K_AUG,), np.float16)
    pad_col[9] = np.float16(4.0)
    pad_col[10] = np.float16(4.0)
    pad_col[11] = np.float16(PAD_DIST / 4.0)
    pad_col[12] = np.float16(0.0)

    in_maps = []
    bases = []
    for c in range(N_CORES):
        b, h = divmod(c, halves)
        lhsT = _aug_rows(a_s[b][h * n_rows:(h + 1) * n_rows], "lhs")
        base = h * n_rows + 64 - W // 2  # global col of local col 0
        bases.append(base)
        rhs = np.repeat(pad_col[:, None], C, axis=1).astype(np.float16)
        g0, g1 = max(base, 0), min(base + C, M)
        rhs[:, g0 - base:g1 - base] = rhs_aug[b][:, g0:g1]
        in_maps.append({"lhsT": lhsT, "rhs": rhs})

    res = run_bass_kernel_spmd(nc, in_maps, list(range(N_CORES)), trace=trace)

    ROWT = n_rows // 128
    COV = 128 * (ROWT - 1) + W
    dist1_s = np.empty((B, N), np.float64)
    dist2_s = np.full((B, M), np.inf, np.float64)
    for c in range(N_CORES):
        b, h = divmod(c, halves)
        o1 = res.results[c]["out1"].astype(np.float64)  # [128, ROWT]
        o2 = res.results[c]["out2"]  # [128, C] fp16 raw col-min accumulator
        dist1_s[b, h * n_rows:(h + 1) * n_rows] = o1.T.reshape(-1)
        colmin = o2.astype(np.float32).min(axis=0).astype(np.float64)
        j_loc = np.arange(C)
        cols = bases[c] + j_loc
        valid = (cols >= 0) & (cols < M) & (j_loc < COV)
        np.minimum.at(dist2_s[b], cols[valid], colmin[valid])
    dist1_s /= DSCALE
    dist2_s /= DSCALE

    # --- exact host patch for at-risk points -----------------------------
    # Window of global tile T (row range [128T,128T+128)): global cols
    # [64 - W/2 + 128T, 64 + W/2 + 128T) intersected with [0, M).
    nt = N // 128
    w_lo = np.maximum(64 - W // 2 + 128 * np.arange(nt), 0)
    w_hi = np.minimum(64 + W // 2 + 128 * np.arange(nt), M)
    for b in range(B):
        ra, rc = ra_s[b], rc_s[b]
        a, cc = a_s[b], c_s[b]
        # coverage radius per sorted row: window covers rc[w_lo[t]..w_hi[t]-1]
        cov1 = np.empty(N)
        for t in range(nt):
            lo = -np.inf if w_lo[t] == 0 else rc[w_lo[t]]
            hi = np.inf if w_hi[t] == M else rc[w_hi[t] - 1]
            rr = ra[128 * t:128 * (t + 1)]
            cov1[128 * t:128 * (t + 1)] = np.minimum(rr - lo, hi - rr)
        # coverage per sorted col: tiles t with w_lo[t] <= j < w_hi[t] form a
        # contiguous range; their rows span a contiguous sorted-row range.
        j_all = np.arange(M)
        ft = np.searchsorted(w_hi - 1, j_all, "left")   # first tile covering j
        lt = np.searchsorted(w_lo, j_all, "right") - 1  # last tile covering j
        lo_r = np.where(ft <= 0, -np.inf, ra[np.minimum(ft * 128, N - 1)])
        hi_r = np.where(lt >= nt - 1, np.inf, ra[np.minimum((lt + 1) * 128 - 1, N - 1)])
        cov2 = np.minimum(rc - lo_r, hi_r - rc)
        risk1 = np.where(dist1_s[b] > cov1 ** 2 * 0.997 - 1e-6)[0]
        risk2 = np.where(dist2_s[b] > cov2 ** 2 * 0.997 - 1e-6)[0]
        if len(risk1):
            d = ((a[risk1][:, None, :].astype(np.float64)
                  - cc[None, :, :].astype(np.float64)) ** 2).sum(-1)
            dist1_s[b][risk1] = d.min(1)
        if len(risk2):
            d = ((cc[risk2][:, None, :].astype(np.float64)
                  - a[None, :, :].astype(np.float64)) ** 2).sum(-1)
            dist2_s[b][risk2] = d.min(1)

    # unsort back to original order
    dist1 = np.empty_like(dist1_s)
    dist2 = np.empty_like(dist2_s)
    for b in range(B):
        dist1[b][perms1[b]] = dist1_s[b]
        dist2[b][perms2[b]] = dist2_s[b]
    return dist1, dist2, res


def _finalize(dist1, dist2):
    dist1 = np.maximum(dist1, 0.0)
    dist2 = np.maximum(dist2, 0.0)
    cd_p = (np.sqrt(dist1).mean(axis=1) + np.sqrt(dist2).mean(axis=1)) / 2.0
    cd_t = dist1.mean(axis=1) + dist2.mean(axis=1)
    p1 = (dist1 < F1_THRESHOLD).mean(axis=1)
    p2 = (dist2 < F1_THRESHOLD).mean(axis=1)
    denom = p1 + p2
    f1 = np.where(denom > 0, 2.0 * p1 * p2 / np.where(denom > 0, denom, 1.0), 0.0)
    return (cd_p.astype(np.float32), cd_t.astype(np.float32),
            f1.astype(np.float32))


def kernel(xyz1, xyz2):
    dist1, dist2, _ = run_device(xyz1, xyz2, trace=False)
    return _finalize(dist1, dist2)
